# revision 3
# baseline (speedup 1.0000x reference)
"""Multi-Head Latent Attention (MLA) forward on 8 Trainium2 NeuronCores, v3.

Problem shapes (hardcoded, self-contained):
  B=2, T=2048, D=2048, H=16, DH=128, DKV=512, DQ=1024, DR=64, fp32 I/O.

Sharding: core ci = b*4 + hg; each core owns one batch element and 4 heads.
Up-projection weights sharded over heads; W_O input-dim sharded, each core
emits a partial (D,T) output summed on host.

Mixed-precision strategy (validated against an element-exact numpy model):
  * Q/K score inputs tolerate fp8-e4m3 noise (softmax diversifies it), so
    the entire Q and K paths run as HOST-FUSED single matmuls
    x @ (W_DQ@W_UQ), x @ (W_DKV@W_UK), x @ W_KR in fp8 DoubleRow mode
    (two 128-row k-tiles per instruction at 0.5 cycles/output element).
    Fusing means each path takes ONE fp8 dot-noise instead of two.
  * The V path, attention probabilities, attention output, and the final
    W_O projection stay bf16 end-to-end: peaked causal attention passes
    V-path noise straight to the output max-error metric.
  * RMSNorm denominators come from dedicated fp8 DoubleRow chains
    (x@W_DQ, x@W_DKV) whose only consumers are sums of squares; the
    rsqrt is a single ACT Rsqrt with all fp8/host scales folded into its
    scale/bias constants.  Norm multipliers are folded into the psum->sbuf
    quantize ops (never a separate pass).
  * Rotate-half is produced by matmul against host-permuted weight columns.
  * Causal masking multiplies exp outputs by a 0/1 bf16 mask on Pool,
    off the PSUM critical path.  Row-sums use an all-ones bf16 lhsT whose
    output lands pre-broadcast across all 128 partitions.
"""

import math

import numpy as np

B, T, D = 2, 2048, 2048
H, DH = 16, 128
DKV, DQ, DR = 512, 1024, 64
ROPE_BASE = 500000.0
EPS = 1e-6
SCALE = 1.0 / math.sqrt(DH + DR)

HL = 4            # heads per core
NCORES = 8
TW = 512          # token tile width
NT = T // TW      # 4 token tiles

# fp8 scale plan (powers of two)
SA = 16.0         # W_DQ / W_DKV norm-chain quantization scale
SFQ = 32.0        # fused Q weight scale
SFK = 32.0        # fused K weight scale
SKR = 16.0        # W_KR scale (columns inside the fused K tensor)
SQ = 2.0          # q/k value scale in fp8
EXP_SCALE = SCALE / (SQ * SQ)
LNA_Q = (SFQ / SQ) ** 2 / (SA * SA * DQ)
LNB_Q = (SFQ / SQ) ** 2 * EPS
LNA_K = (SFK / SQ) ** 2 / (SA * SA * DKV)
LNB_K = (SFK / SQ) ** 2 * EPS
KR_C = SQ / SKR
V_C = SFK / SQ    # v multiplier: nkv_row * V_C = true rsqrt(ms+eps)

_CACHE: dict = {}
LAST_EXEC_NS = None


def _build():
    from contextlib import ExitStack

    import concourse.mybir as mybir
    import concourse.tile as tile
    from concourse.bacc import Bacc

    f32 = mybir.dt.float32
    bf16 = mybir.dt.bfloat16
    fp8 = mybir.dt.float8e4
    AF = mybir.ActivationFunctionType
    DRM = mybir.MatmulPerfMode.DoubleRow
    ALU = mybir.AluOpType

    nc = Bacc("TRN2")

    # fp8 tensors arrive pre-paired: [128, npair, 2, N], row = pr*256+two*128+p
    x8_d = nc.dram_tensor("x8", (128, 8, 2, T), fp8, kind="ExternalInput")
    xb_d = nc.dram_tensor("xb", (128, 16, T), bf16, kind="ExternalInput")
    wdq_d = nc.dram_tensor("wdq", (128, 8, 2, DQ), fp8, kind="ExternalInput")
    wkv_d = nc.dram_tensor("wkv", (128, 8, 2, DKV), fp8, kind="ExternalInput")
    wfq_d = nc.dram_tensor("wfq", (128, 8, 2, 1024), fp8, kind="ExternalInput")
    wfk_d = nc.dram_tensor("wfk", (128, 8, 2, 640), fp8, kind="ExternalInput")
    wfv_d = nc.dram_tensor("wfv", (128, 16, 512), bf16, kind="ExternalInput")
    wo_d = nc.dram_tensor("wo", (128, 4, D), bf16, kind="ExternalInput")
    cos_d = nc.dram_tensor("costab", (128, T), bf16, kind="ExternalInput")
    sin_d = nc.dram_tensor("sintab", (128, T), bf16, kind="ExternalInput")
    out_d = nc.dram_tensor("final_t", (D, T), f32, kind="ExternalOutput")

    with tile.TileContext(nc) as tc, ExitStack() as ctx:
        pers = ctx.enter_context(tc.tile_pool(name="pers", bufs=1))

        # ---------- persistent weights (DMA in use order) ----------
        wdq_sb = pers.tile([128, 8, 2, DQ], fp8, tag="wdq")
        wkv_sb = pers.tile([128, 8, 2, DKV], fp8, tag="wkv")
        wfq_sb = pers.tile([128, 8, 2, 1024], fp8, tag="wfq")
        wfk_sb = pers.tile([128, 8, 2, 640], fp8, tag="wfk")
        wfv_sb = pers.tile([128, 16, 512], bf16, tag="wfv")
        wo_sb = pers.tile([128, 4, D], bf16, tag="wo")
        cos_sb = pers.tile([128, T], bf16, tag="cos")
        sin_sb = pers.tile([128, T], bf16, tag="sin")

        # ---------- persistent activations ----------
        # kall slots: 0-3 content per head, 4 [kr;0], 5 [0;kr]   (fp8, SQ*khat)
        kall_sb = pers.tile([128, 6, T], fp8, tag="kall")
        # v token-major bf16: [128 tok, kt 16, head, dh]
        v_sb = pers.tile([128, 16, HL, DH], bf16, tag="v")
        # attention out feature-major bf16: [128 dh, head, T]
        ao_sb = pers.tile([128, HL, T], bf16, tag="ao")

        # ---------- constants ----------
        onesb = pers.tile([128, 128], bf16, tag="onesb")
        twosb = pers.tile([128, 1], bf16, tag="twosb")
        lnbq_sb = pers.tile([1, 1], f32, tag="lnbq")
        lnbk_sb = pers.tile([1, 1], f32, tag="lnbk")
        id1_sb = pers.tile([1, 1], f32, tag="id1")
        mask01 = pers.tile([128, 4, TW], bf16, tag="mask01")

        def prologue():
            nc.sync.dma_start(out=wdq_sb[:, :, :, 0:512], in_=wdq_d[:, :, :, 0:512])
            nc.sync.dma_start(out=wdq_sb[:, :, :, 512:1024], in_=wdq_d[:, :, :, 512:1024])
            nc.sync.dma_start(out=wfq_sb, in_=wfq_d[:, :, :, :])
            nc.sync.dma_start(out=wkv_sb, in_=wkv_d[:, :, :, :])
            nc.sync.dma_start(out=wfk_sb, in_=wfk_d[:, :, :, :])
            nc.sync.dma_start(out=wfv_sb, in_=wfv_d[:, :, :])
            nc.sync.dma_start(out=cos_sb, in_=cos_d[:, :])
            nc.sync.dma_start(out=sin_sb, in_=sin_d[:, :])
            nc.sync.dma_start(out=wo_sb, in_=wo_d[:, :, :])
            nc.gpsimd.memset(kall_sb[:, 4:6, :], 0.0)
            nc.gpsimd.memset(onesb, 1.0)
            nc.gpsimd.memset(twosb, 2.0)
            nc.vector.memset(lnbq_sb, LNB_Q)
            nc.vector.memset(lnbk_sb, LNB_K)
            nc.vector.memset(id1_sb, 1.0)
            nc.gpsimd.memset(mask01, 1.0)
            for j in range(4):
                nc.gpsimd.affine_select(
                    out=mask01[:, j, :], in_=mask01[:, j, :],
                    compare_op=ALU.is_ge, fill=0.0,
                    base=-128 * j, pattern=[[1, TW]], channel_multiplier=-1)


        # ---------- pools ----------
        x8in = ctx.enter_context(tc.tile_pool(name="x8in", bufs=2))
        xbin = ctx.enter_context(tc.tile_pool(name="xbin", bufs=1))
        qall_p = ctx.enter_context(tc.tile_pool(name="qall", bufs=2))
        sq_p = ctx.enter_context(tc.tile_pool(name="sq", bufs=1))
        nrm_p = ctx.enter_context(tc.tile_pool(name="nrm", bufs=1))
        rope_p = ctx.enter_context(tc.tile_pool(name="rope", bufs=1))
        pt_p = ctx.enter_context(tc.tile_pool(name="pt", bufs=4))
        rbc_p = ctx.enter_context(tc.tile_pool(name="rbc", bufs=1))
        od_p = ctx.enter_context(tc.tile_pool(name="od", bufs=3))

        psAB = ctx.enter_context(tc.tile_pool(name="psAB", bufs=4, space="PSUM"))
        psS = ctx.enter_context(tc.tile_pool(name="psS", bufs=2, space="PSUM"))
        psO = ctx.enter_context(tc.tile_pool(name="psO", bufs=1, space="PSUM"))
        psR = ctx.enter_context(tc.tile_pool(name="psR", bufs=1, space="PSUM"))

        def dr_chain(out_ps, w_sb, col0, x_sb, npair):
            for pr in range(npair):
                nc.tensor.matmul(
                    out_ps,
                    lhsT=w_sb[:, pr, :, col0:col0 + 128],
                    rhs=x_sb[:, pr, :, :],
                    start=(pr == 0), stop=(pr == npair - 1),
                    perf_mode=DRM)

        # ---------- main loop with cross-phase interleaving ----------
        x8_tiles = {}
        xb_tiles = {}

        def load_x(t, xb_too=True):
            ts = slice(t * TW, (t + 1) * TW)
            if t < NT and t not in x8_tiles:
                xs = x8in.tile([128, 8, 2, TW], fp8, tag="x8")
                nc.sync.dma_start(out=xs, in_=x8_d[:, :, :, ts])
                x8_tiles[t] = xs
            if xb_too and t < NT and t not in xb_tiles:
                xbs = xbin.tile([128, 16, TW], bf16, tag="xb")
                nc.sync.dma_start(out=xbs, in_=xb_d[:, :, ts])
                xb_tiles[t] = xbs

        def emit_A(t, dfill):
            """norm chains + kr; returns (nq_bc, nkv_bc, nkvT) tiles"""
            ts = slice(t * TW, (t + 1) * TW)
            x_sb = x8_tiles[t]
            di = [0]

            def dpop():
                if di[0] < len(dfill):
                    dfill[di[0]]()
                    di[0] += 1
            sum_q = psS.tile([128, TW], f32, tag="s")
            sum_k = psS.tile([128, TW], f32, tag="s")
            sq_q = []
            for mc in range(12):
                mm = psAB.tile([128, TW], f32, tag="m")
                if mc < 8:
                    dr_chain(mm, wdq_sb, mc * 128, x_sb, 8)
                else:
                    dr_chain(mm, wkv_sb, (mc - 8) * 128, x_sb, 8)
                sq = sq_p.tile([128, TW], bf16, tag=f"sq{mc % 2}")
                nc.scalar.square(sq, mm)
                sq_q.append(sq)
                dpop()
                if mc >= 1:
                    k = mc - 1
                    sqd = sq_q[k]
                    if k < 8:
                        nc.tensor.matmul(sum_q[0:1, :], lhsT=onesb[:, 0:1],
                                         rhs=sqd, start=(k == 0),
                                         stop=(k == 7))
                    else:
                        nc.tensor.matmul(sum_k[0:1, :],
                                         lhsT=twosb[:, 0:1], rhs=sqd,
                                         start=(k == 8), stop=False)
            sqd = sq_q[11]
            nc.tensor.matmul(sum_k[0:1, :], lhsT=twosb[:, 0:1],
                             rhs=sqd, start=False, stop=True)
            # kr chain (no norm dependency)
            mm = psAB.tile([128, TW], f32, tag="m")
            dr_chain(mm, wfk_sb, 512, x_sb, 8)
            krt = rope_p.tile([128, TW], bf16, tag="krt")
            nc.scalar.copy(krt, mm)
            t1 = rope_p.tile([128, TW], f32, tag="t1")
            t2 = rope_p.tile([128, TW], f32, tag="t2")
            nc.vector.scalar_tensor_tensor(
                t1[0:64, :], krt[0:64, :], KR_C, cos_sb[0:64, ts],
                op0=ALU.mult, op1=ALU.mult)
            nc.vector.scalar_tensor_tensor(
                t2[0:64, :], krt[64:128, :], KR_C, sin_sb[64:128, ts],
                op0=ALU.mult, op1=ALU.mult)
            nc.gpsimd.tensor_add(kall_sb[0:64, 4, ts], t1[0:64, :],
                                 t2[0:64, :])
            nc.vector.tensor_copy(kall_sb[64:128, 5, ts],
                                  kall_sb[0:64, 4, ts])
            while di[0] < len(dfill):
                dfill[di[0]]()
                di[0] += 1
            # norms: Ln,Ln then Exp,Exp keeps table switches minimal
            n2_row = nrm_p.tile([1, 2 * TW], f32, tag="n2")
            nq_row = n2_row[:, 0:TW]
            nkv_row = n2_row[:, TW:2 * TW]
            nc.scalar.activation(nq_row, sum_q[0:1, :], func=AF.Ln,
                                 scale=LNA_Q, bias=lnbq_sb)
            nc.scalar.activation(nkv_row, sum_k[0:1, :], func=AF.Ln,
                                 scale=LNA_Q, bias=lnbq_sb)
            nc.scalar.activation(n2_row, n2_row, func=AF.Exp, scale=-0.5)
            nq_bc = nrm_p.tile([128, TW], f32, tag="nqbc")
            nc.gpsimd.partition_broadcast(nq_bc, nq_row)
            nkv_bc = nrm_p.tile([128, TW], f32, tag="nkvbc")
            nc.gpsimd.partition_broadcast(nkv_bc, nkv_row)
            x2 = psO.tile([128, TW], f32, tag="o")
            for j in range(4):
                nc.tensor.transpose(x2[:, j:j + 1],
                                    nkv_row[:, j * 128:(j + 1) * 128],
                                    id1_sb)
            nkvT = nrm_p.tile([128, 4], f32, tag="nkvT")
            nc.vector.tensor_copy(nkvT, x2[:, 0:4])
            return nq_bc, nkv_bc, nkvT

        def b_units(t, nq_bc, nkv_bc, nkvT):
            """list of closures, each one PE chain of phase B for tile t"""
            ts = slice(t * TW, (t + 1) * TW)
            x_sb = x8_tiles[t]
            xb_sb = xb_tiles[t]
            units = []
            holder = {}
            qall_sb = qall_p.tile([128, 6, TW], fp8, tag="qall")

            def qc_unit(h):
                def f():
                    mm = psAB.tile([128, TW], f32, tag="m")
                    dr_chain(mm, wfq_sb, h * 128, x_sb, 8)
                    nc.vector.tensor_mul(qall_sb[:, h, :], mm, nq_bc)
                return f

            def qprep_unit():
                def f():
                    cosn = rope_p.tile([128, TW], f32, tag="cosn")
                    sinn = rope_p.tile([128, TW], f32, tag="sinn")
                    nc.vector.tensor_mul(cosn, cos_sb[:, ts], nq_bc)
                    nc.vector.tensor_mul(sinn, sin_sb[:, ts], nq_bc)
                    holder["cosn"] = cosn
                    holder["sinn"] = sinn
                return f

            def qr_unit(j):
                def f():
                    qr_ps = psAB.tile([128, TW], f32, tag="m")
                    dr_chain(qr_ps, wfq_sb, 512 + j * 128, x_sb, 8)
                    rot_ps = psAB.tile([128, TW], f32, tag="m")
                    dr_chain(rot_ps, wfq_sb, 768 + j * 128, x_sb, 8)
                    t1 = rope_p.tile([128, TW], f32, tag="t1")
                    t2 = rope_p.tile([128, TW], f32, tag="t2")
                    nc.vector.tensor_mul(t1, qr_ps, holder["cosn"])
                    nc.vector.tensor_mul(t2, rot_ps, holder["sinn"])
                    nc.gpsimd.tensor_add(qall_sb[:, 4 + j, :], t1, t2)
                return f

            def k_unit(h):
                def f():
                    mm = psAB.tile([128, TW], f32, tag="m")
                    dr_chain(mm, wfk_sb, h * 128, x_sb, 8)
                    nc.vector.tensor_mul(kall_sb[:, h, ts], mm, nkv_bc)
                return f

            def v_unit(tc4):
                def f():
                    mm = psAB.tile([128, TW], f32, tag="m")
                    for kt in range(16):
                        nc.tensor.matmul(
                            mm,
                            lhsT=xb_sb[:, kt, tc4 * 128:(tc4 + 1) * 128],
                            rhs=wfv_sb[:, kt, :],
                            start=(kt == 0), stop=(kt == 15))
                    nc.vector.tensor_scalar(
                        v_sb[:, t * 4 + tc4, :, :].rearrange(
                            "p h d -> p (h d)"),
                        mm, nkvT[:, tc4:tc4 + 1], V_C,
                        op0=ALU.mult, op1=ALU.mult)
                return f

            units = [qc_unit(h) for h in range(HL)]
            units.append(qprep_unit())
            units += [qr_unit(j) for j in range(2)]
            units += [k_unit(h) for h in range(HL)]
            units += [v_unit(tc4) for tc4 in range(4)]
            return units, qall_sb

        qall_tiles = {}

        def emit_C(tq, fillers):
            """attention for query tile tq with PE filler units woven in"""
            ts = slice(tq * TW, (tq + 1) * TW)
            qall_sb = qall_tiles[tq]
            nkt = 4 * (tq + 1)
            PIPE = 2
            fi = 0
            for h in range(HL):
                qsl = 4 + h // 2 - h
                ksl = 4 + h % 2 - h
                ao_ps = psO.tile([128, TW], f32, tag="o")
                rs_ps = psR.tile([128, TW], f32, tag="r")
                pts = {}
                for kt in range(nkt + PIPE):
                    if kt < nkt:
                        s_ps = psS.tile([128, TW], f32, tag="s")
                        nc.tensor.matmul(
                            s_ps,
                            lhsT=kall_sb[:, h::ksl, kt * 128:(kt + 1) * 128],
                            rhs=qall_sb[:, h::qsl, :],
                            start=True, stop=True, perf_mode=DRM)
                        pt = pt_p.tile([128, TW], bf16, tag="pt")
                        nc.scalar.activation(pt, s_ps, func=AF.Exp,
                                             scale=EXP_SCALE)
                        j = kt - 4 * tq
                        if j >= 0:
                            nc.vector.tensor_mul(pt, pt, mask01[:, j, :])
                        pts[kt] = pt
                    if kt % 3 == 2 and fi < len(fillers):
                        fillers[fi]()
                        fi += 1
                    kd = kt - PIPE
                    if kd >= 0:
                        pt = pts.pop(kd)
                        nc.tensor.matmul(
                            ao_ps, lhsT=v_sb[:, kd, h, :], rhs=pt,
                            start=(kd == 0), stop=(kd == nkt - 1))
                        nc.tensor.matmul(
                            rs_ps, lhsT=onesb, rhs=pt,
                            start=(kd == 0), stop=(kd == nkt - 1))
                rbc = rbc_p.tile([128, TW], f32, tag="rbc")
                nc.vector.reciprocal(rbc, rs_ps)
                nc.vector.tensor_mul(ao_sb[:, h, ts], ao_ps, rbc)
            for f in fillers[fi:]:
                f()

        def d_units(tq):
            ts = slice(tq * TW, (tq + 1) * TW)

            def unit(dc):
                def f():
                    mm = psAB.tile([128, TW], f32, tag="m")
                    for kt4 in range(4):
                        nc.tensor.matmul(
                            mm, lhsT=wo_sb[:, kt4, dc * 128:(dc + 1) * 128],
                            rhs=ao_sb[:, kt4, ts],
                            start=(kt4 == 0), stop=(kt4 == 3))
                    o_sb = od_p.tile([128, TW], f32, tag="od")
                    if dc % 2 == 0:
                        nc.scalar.copy(o_sb, mm)
                    else:
                        nc.vector.tensor_copy(o_sb, mm)
                    nc.gpsimd.dma_start(
                        out=out_d[dc * 128:(dc + 1) * 128, ts], in_=o_sb)
                return f
            return [unit(dc) for dc in range(16)]

        load_x(0, xb_too=False)
        prologue()
        load_x(0)
        load_x(1)
        pend_d = []
        for t in range(NT):
            load_x(t + 1)
            nq_bc, nkv_bc, nkvT = emit_A(t, pend_d)
            units, qall_sb = b_units(t, nq_bc, nkv_bc, nkvT)
            qall_tiles[t] = qall_sb
            if t == 0:
                for f in units:
                    f()
            else:
                emit_C(t - 1, units)
                pend_d = d_units(t - 1)
        emit_C(NT - 1, pend_d)
        for f in d_units(NT - 1):
            f()

    nc.finalize()
    return nc


def _rope_tables():
    inv_freq = (1.0 / (ROPE_BASE ** (np.arange(0, DR, 2, dtype=np.float64)
                                     / DR)))
    tt = np.arange(T, dtype=np.float64)
    freqs = np.outer(tt, inv_freq)
    emb = np.concatenate([freqs, freqs], axis=-1)
    cos = np.cos(emb).T
    sin = np.sin(emb).T
    cos128 = np.ascontiguousarray(np.concatenate([cos, cos], 0))
    sin128 = np.ascontiguousarray(np.concatenate([sin, sin], 0))
    return cos128, sin128


def _pair_rows(w):
    """(K, N) -> (128, K//256, 2, N) with row index = pr*256 + two*128 + p"""
    K, N = w.shape
    return np.ascontiguousarray(
        w.reshape(K // 256, 2, 128, N).transpose(2, 0, 1, 3))


def _rows(w):
    """(K, N) -> (128, K//128, N) with row index = c*128 + p"""
    K, N = w.shape
    return np.ascontiguousarray(w.reshape(K // 128, 128, N).transpose(1, 0, 2))


def _rot_cols(w, dr):
    K, N = w.shape
    wh = w.reshape(K, N // dr, dr)
    lo, hi = wh[:, :, :dr // 2], wh[:, :, dr // 2:]
    return np.concatenate([-hi, lo], axis=2).reshape(K, N)


def _to_fp8(a):
    import ml_dtypes
    return np.ascontiguousarray(a).astype(ml_dtypes.float8_e4m3)


def _to_bf16(a):
    import ml_dtypes
    return np.ascontiguousarray(a).astype(ml_dtypes.bfloat16)


def kernel(x, W_DQ, W_UQ, W_QR, W_DKV, W_UK, W_UV, W_KR, W_O,
           q_norm_w, kv_norm_w):
    global LAST_EXEC_NS
    from concourse.bass_utils import run_bass_kernel_spmd

    x = np.asarray(x, dtype=np.float64)
    W_DQ = np.asarray(W_DQ, np.float64)
    W_UQ = np.asarray(W_UQ, np.float64)
    W_QR = np.asarray(W_QR, np.float64)
    W_DKV = np.asarray(W_DKV, np.float64)
    W_UK = np.asarray(W_UK, np.float64)
    W_UV = np.asarray(W_UV, np.float64)
    W_KR = np.asarray(W_KR, np.float64)
    W_O = np.asarray(W_O, np.float64)
    q_norm_w = np.asarray(q_norm_w, np.float64)
    kv_norm_w = np.asarray(kv_norm_w, np.float64)

    # fold norm weights into up-projections (host, f64)
    wuq_f = W_UQ * q_norm_w[:, None]
    wqr_f = W_QR * q_norm_w[:, None]
    wuk_f = W_UK * kv_norm_w[:, None]
    wuv_f = W_UV * kv_norm_w[:, None]

    cos128, sin128 = _rope_tables()
    cos_b = _to_bf16(cos128)
    sin_b = _to_bf16(sin128)

    wuq_h = wuq_f.reshape(DQ, H, DH)
    wqr_h = wqr_f.reshape(DQ, H, DR)
    wuk_h = wuk_f.reshape(DKV, H, DH)
    wuv_h = wuv_f.reshape(DKV, H, DH)
    wo_h = W_O.reshape(H, DH, D)

    wdq_p = _to_fp8(_pair_rows(W_DQ * SA))
    wkv_p = _to_fp8(_pair_rows(W_DKV * SA))
    wkr_cat = np.concatenate(
        [W_KR * SKR, _rot_cols(W_KR.reshape(D, DR), DR) * SKR], axis=1)

    in_maps = []
    for ci in range(NCORES):
        b, hg = divmod(ci, H // HL)
        hsl = slice(hg * HL, (hg + 1) * HL)
        wuq_s = wuq_h[:, hsl].reshape(DQ, HL * DH)
        wqr_s = wqr_h[:, hsl].reshape(DQ, HL * DR)
        wfq = np.concatenate(
            [W_DQ @ wuq_s, W_DQ @ wqr_s, W_DQ @ _rot_cols(wqr_s, DR)],
            axis=1) * SFQ                                    # (D, 1024)
        wuk_s = wuk_h[:, hsl].reshape(DKV, HL * DH)
        wfk = np.concatenate(
            [W_DKV @ wuk_s * SFK, wkr_cat], axis=1)          # (D, 640)
        wfv = W_DKV @ wuv_h[:, hsl].reshape(DKV, HL * DH)    # (D, 512)
        in_maps.append({
            "x8": _to_fp8(_pair_rows(x[b].T)),
            "xb": _to_bf16(_rows(x[b].T)),
            "wdq": wdq_p,
            "wkv": wkv_p,
            "wfq": _to_fp8(_pair_rows(wfq)),
            "wfk": _to_fp8(_pair_rows(wfk)),
            "wfv": _to_bf16(_rows(wfv)),
            "wo": _to_bf16(_rows(wo_h[hsl].reshape(HL * DH, D))),
            "costab": cos_b,
            "sintab": sin_b,
        })

    if "nc" not in _CACHE:
        _CACHE["nc"] = _build()
    nc = _CACHE["nc"]

    import os as _os
    _trace = _os.environ.get("MLA_TRACE") == "1"
    res = run_bass_kernel_spmd(
        nc, in_maps, core_ids=list(range(NCORES)), trace=_trace)
    LAST_EXEC_NS = res.exec_time_ns
    outs = [res.results[ci]["final_t"] for ci in range(NCORES)]

    out = np.zeros((B, T, D), np.float32)
    for ci in range(NCORES):
        b = ci // (H // HL)
        out[b] += outs[ci].T
    return out


# revision 4
# speedup vs baseline: 1.0035x; 1.0035x over previous
"""Multi-Head Latent Attention (MLA) forward on 8 Trainium2 NeuronCores, v3.

Problem shapes (hardcoded, self-contained):
  B=2, T=2048, D=2048, H=16, DH=128, DKV=512, DQ=1024, DR=64, fp32 I/O.

Sharding: core ci = b*4 + hg; each core owns one batch element and 4 heads.
Up-projection weights sharded over heads; W_O input-dim sharded, each core
emits a partial (D,T) output summed on host.

Mixed-precision strategy (validated against an element-exact numpy model):
  * Q/K score inputs tolerate fp8-e4m3 noise (softmax diversifies it), so
    the entire Q and K paths run as HOST-FUSED single matmuls
    x @ (W_DQ@W_UQ), x @ (W_DKV@W_UK), x @ W_KR in fp8 DoubleRow mode
    (two 128-row k-tiles per instruction at 0.5 cycles/output element).
    Fusing means each path takes ONE fp8 dot-noise instead of two.
  * The V path, attention probabilities, attention output, and the final
    W_O projection stay bf16 end-to-end: peaked causal attention passes
    V-path noise straight to the output max-error metric.
  * RMSNorm denominators come from dedicated fp8 DoubleRow chains
    (x@W_DQ, x@W_DKV) whose only consumers are sums of squares; the
    rsqrt is a single ACT Rsqrt with all fp8/host scales folded into its
    scale/bias constants.  Norm multipliers are folded into the psum->sbuf
    quantize ops (never a separate pass).
  * Rotate-half is produced by matmul against host-permuted weight columns.
  * Causal masking multiplies exp outputs by a 0/1 bf16 mask on Pool,
    off the PSUM critical path.  Row-sums use an all-ones bf16 lhsT whose
    output lands pre-broadcast across all 128 partitions.
"""

import math

import numpy as np

B, T, D = 2, 2048, 2048
H, DH = 16, 128
DKV, DQ, DR = 512, 1024, 64
ROPE_BASE = 500000.0
EPS = 1e-6
SCALE = 1.0 / math.sqrt(DH + DR)

HL = 4            # heads per core
NCORES = 8
TW = 512          # token tile width
NT = T // TW      # 4 token tiles

# fp8 scale plan (powers of two)
SA = 16.0         # W_DQ / W_DKV norm-chain quantization scale
SFQ = 32.0        # fused Q weight scale
SFK = 32.0        # fused K weight scale
SKR = 16.0        # W_KR scale (columns inside the fused K tensor)
SQ = 2.0          # q/k value scale in fp8
EXP_SCALE = SCALE / (SQ * SQ)
LNA_Q = (SFQ / SQ) ** 2 / (SA * SA * DQ)
LNB_Q = (SFQ / SQ) ** 2 * EPS
LNA_K = (SFK / SQ) ** 2 / (SA * SA * DKV)
LNB_K = (SFK / SQ) ** 2 * EPS
KR_C = SQ / SKR
V_C = SFK / SQ    # v multiplier: nkv_row * V_C = true rsqrt(ms+eps)

_CACHE: dict = {}
LAST_EXEC_NS = None


def _build():
    from contextlib import ExitStack

    import concourse.mybir as mybir
    import concourse.tile as tile
    from concourse.bacc import Bacc

    f32 = mybir.dt.float32
    bf16 = mybir.dt.bfloat16
    fp8 = mybir.dt.float8e4
    AF = mybir.ActivationFunctionType
    DRM = mybir.MatmulPerfMode.DoubleRow
    ALU = mybir.AluOpType

    nc = Bacc("TRN2")

    # fp8 tensors arrive pre-paired: [128, npair, 2, N], row = pr*256+two*128+p
    x8_d = nc.dram_tensor("x8", (128, 8, 2, T), fp8, kind="ExternalInput")
    xb_d = nc.dram_tensor("xb", (128, 16, T), bf16, kind="ExternalInput")
    wdq_d = nc.dram_tensor("wdq", (128, 8, 2, DQ), fp8, kind="ExternalInput")
    wkv_d = nc.dram_tensor("wkv", (128, 8, 2, DKV), fp8, kind="ExternalInput")
    wfq_d = nc.dram_tensor("wfq", (128, 8, 2, 1024), fp8, kind="ExternalInput")
    wfk_d = nc.dram_tensor("wfk", (128, 8, 2, 640), fp8, kind="ExternalInput")
    wfv_d = nc.dram_tensor("wfv", (128, 16, 512), bf16, kind="ExternalInput")
    wo_d = nc.dram_tensor("wo", (128, 4, D), bf16, kind="ExternalInput")
    cos_d = nc.dram_tensor("costab", (128, T), bf16, kind="ExternalInput")
    sin_d = nc.dram_tensor("sintab", (128, T), bf16, kind="ExternalInput")
    out_d = nc.dram_tensor("final_t", (D, T), f32, kind="ExternalOutput")

    with tile.TileContext(nc) as tc, ExitStack() as ctx:
        pers = ctx.enter_context(tc.tile_pool(name="pers", bufs=1))

        # ---------- persistent weights (DMA in use order) ----------
        wdq_sb = pers.tile([128, 8, 2, DQ], fp8, tag="wdq")
        wkv_sb = pers.tile([128, 8, 2, DKV], fp8, tag="wkv")
        wfq_sb = pers.tile([128, 8, 2, 1024], fp8, tag="wfq")
        wfk_sb = pers.tile([128, 8, 2, 640], fp8, tag="wfk")
        wfv_sb = pers.tile([128, 16, 512], bf16, tag="wfv")
        wo_sb = pers.tile([128, 4, D], bf16, tag="wo")
        cos_sb = pers.tile([128, T], bf16, tag="cos")
        sin_sb = pers.tile([128, T], bf16, tag="sin")

        # ---------- persistent activations ----------
        # kall slots: 0-3 content per head, 4 [kr;0], 5 [0;kr]   (fp8, SQ*khat)
        kall_sb = pers.tile([128, 6, T], fp8, tag="kall")
        # v token-major bf16: [128 tok, kt 16, head, dh]
        v_sb = pers.tile([128, 16, HL, DH], bf16, tag="v")
        # attention out feature-major bf16: [128 dh, head, T]
        ao_sb = pers.tile([128, HL, T], bf16, tag="ao")

        # ---------- constants ----------
        onesb = pers.tile([128, 128], bf16, tag="onesb")
        twosb = pers.tile([128, 1], bf16, tag="twosb")
        lnbq_sb = pers.tile([1, 1], f32, tag="lnbq")
        lnbk_sb = pers.tile([1, 1], f32, tag="lnbk")
        id1_sb = pers.tile([1, 1], f32, tag="id1")
        mask01 = pers.tile([128, 4, TW], bf16, tag="mask01")

        def prologue():
            nc.sync.dma_start(out=wdq_sb[:, :, :, 512:1024], in_=wdq_d[:, :, :, 512:1024])
            nc.sync.dma_start(out=wfq_sb, in_=wfq_d[:, :, :, :])
            nc.sync.dma_start(out=wkv_sb, in_=wkv_d[:, :, :, :])
            nc.sync.dma_start(out=wfk_sb, in_=wfk_d[:, :, :, :])
            nc.sync.dma_start(out=wfv_sb, in_=wfv_d[:, :, :])
            nc.sync.dma_start(out=cos_sb, in_=cos_d[:, :])
            nc.sync.dma_start(out=sin_sb, in_=sin_d[:, :])
            nc.sync.dma_start(out=wo_sb, in_=wo_d[:, :, :])
            nc.gpsimd.memset(kall_sb[:, 4:6, :], 0.0)
            nc.gpsimd.memset(onesb, 1.0)
            nc.gpsimd.memset(twosb, 2.0)
            nc.vector.memset(lnbq_sb, LNB_Q)
            nc.vector.memset(lnbk_sb, LNB_K)
            nc.vector.memset(id1_sb, 1.0)
            nc.gpsimd.memset(mask01, 1.0)
            for j in range(4):
                nc.gpsimd.affine_select(
                    out=mask01[:, j, :], in_=mask01[:, j, :],
                    compare_op=ALU.is_ge, fill=0.0,
                    base=-128 * j, pattern=[[1, TW]], channel_multiplier=-1)


        # ---------- pools ----------
        x8in = ctx.enter_context(tc.tile_pool(name="x8in", bufs=2))
        xbin = ctx.enter_context(tc.tile_pool(name="xbin", bufs=1))
        qall_p = ctx.enter_context(tc.tile_pool(name="qall", bufs=2))
        sq_p = ctx.enter_context(tc.tile_pool(name="sq", bufs=1))
        nrm_p = ctx.enter_context(tc.tile_pool(name="nrm", bufs=1))
        rope_p = ctx.enter_context(tc.tile_pool(name="rope", bufs=1))
        pt_p = ctx.enter_context(tc.tile_pool(name="pt", bufs=4))
        rbc_p = ctx.enter_context(tc.tile_pool(name="rbc", bufs=1))
        od_p = ctx.enter_context(tc.tile_pool(name="od", bufs=3))

        psAB = ctx.enter_context(tc.tile_pool(name="psAB", bufs=4, space="PSUM"))
        psS = ctx.enter_context(tc.tile_pool(name="psS", bufs=2, space="PSUM"))
        psO = ctx.enter_context(tc.tile_pool(name="psO", bufs=1, space="PSUM"))
        psR = ctx.enter_context(tc.tile_pool(name="psR", bufs=1, space="PSUM"))

        def dr_chain(out_ps, w_sb, col0, x_sb, npair):
            for pr in range(npair):
                nc.tensor.matmul(
                    out_ps,
                    lhsT=w_sb[:, pr, :, col0:col0 + 128],
                    rhs=x_sb[:, pr, :, :],
                    start=(pr == 0), stop=(pr == npair - 1),
                    perf_mode=DRM)

        # ---------- main loop with cross-phase interleaving ----------
        x8_tiles = {}
        xb_tiles = {}

        def load_x(t, xb_too=True):
            ts = slice(t * TW, (t + 1) * TW)
            if t < NT and t not in x8_tiles:
                xs = x8in.tile([128, 8, 2, TW], fp8, tag="x8")
                nc.sync.dma_start(out=xs, in_=x8_d[:, :, :, ts])
                x8_tiles[t] = xs
            if xb_too and t < NT and t not in xb_tiles:
                xbs = xbin.tile([128, 16, TW], bf16, tag="xb")
                nc.sync.dma_start(out=xbs, in_=xb_d[:, :, ts])
                xb_tiles[t] = xbs

        def emit_A(t, dfill):
            """norm chains + kr; returns (nq_bc, nkv_bc, nkvT) tiles"""
            ts = slice(t * TW, (t + 1) * TW)
            x_sb = x8_tiles[t]
            di = [0]

            def dpop():
                if di[0] < len(dfill):
                    dfill[di[0]]()
                    di[0] += 1
            sum_q = psS.tile([128, TW], f32, tag="s")
            sum_k = psS.tile([128, TW], f32, tag="s")
            sq_q = []
            for mc in range(12):
                mm = psAB.tile([128, TW], f32, tag="m")
                if mc < 8:
                    dr_chain(mm, wdq_sb, mc * 128, x_sb, 8)
                else:
                    dr_chain(mm, wkv_sb, (mc - 8) * 128, x_sb, 8)
                sq = sq_p.tile([128, TW], bf16, tag=f"sq{mc % 2}")
                nc.scalar.square(sq, mm)
                sq_q.append(sq)
                dpop()
                if mc >= 1:
                    k = mc - 1
                    sqd = sq_q[k]
                    if k < 8:
                        nc.tensor.matmul(sum_q[0:1, :], lhsT=onesb[:, 0:1],
                                         rhs=sqd, start=(k == 0),
                                         stop=(k == 7))
                    else:
                        nc.tensor.matmul(sum_k[0:1, :],
                                         lhsT=twosb[:, 0:1], rhs=sqd,
                                         start=(k == 8), stop=False)
            sqd = sq_q[11]
            nc.tensor.matmul(sum_k[0:1, :], lhsT=twosb[:, 0:1],
                             rhs=sqd, start=False, stop=True)
            # kr chain (no norm dependency)
            mm = psAB.tile([128, TW], f32, tag="m")
            dr_chain(mm, wfk_sb, 512, x_sb, 8)
            krt = rope_p.tile([128, TW], bf16, tag="krt")
            nc.scalar.copy(krt, mm)
            t1 = rope_p.tile([128, TW], f32, tag="t1")
            t2 = rope_p.tile([128, TW], f32, tag="t2")
            nc.vector.scalar_tensor_tensor(
                t1[0:64, :], krt[0:64, :], KR_C, cos_sb[0:64, ts],
                op0=ALU.mult, op1=ALU.mult)
            nc.vector.scalar_tensor_tensor(
                t2[0:64, :], krt[64:128, :], KR_C, sin_sb[64:128, ts],
                op0=ALU.mult, op1=ALU.mult)
            nc.gpsimd.tensor_add(kall_sb[0:64, 4, ts], t1[0:64, :],
                                 t2[0:64, :])
            nc.vector.tensor_copy(kall_sb[64:128, 5, ts],
                                  kall_sb[0:64, 4, ts])
            while di[0] < len(dfill):
                dfill[di[0]]()
                di[0] += 1
            # norms: Ln,Ln then Exp,Exp keeps table switches minimal
            n2_row = nrm_p.tile([1, 2 * TW], f32, tag="n2")
            nq_row = n2_row[:, 0:TW]
            nkv_row = n2_row[:, TW:2 * TW]
            nc.scalar.activation(nq_row, sum_q[0:1, :], func=AF.Ln,
                                 scale=LNA_Q, bias=lnbq_sb)
            nc.scalar.activation(nkv_row, sum_k[0:1, :], func=AF.Ln,
                                 scale=LNA_Q, bias=lnbq_sb)
            nc.scalar.activation(n2_row, n2_row, func=AF.Exp, scale=-0.5)
            nq_bc = nrm_p.tile([128, TW], f32, tag="nqbc")
            nc.gpsimd.partition_broadcast(nq_bc, nq_row)
            nkv_bc = nrm_p.tile([128, TW], f32, tag="nkvbc")
            nc.gpsimd.partition_broadcast(nkv_bc, nkv_row)
            x2 = psO.tile([128, TW], f32, tag="o")
            for j in range(4):
                nc.tensor.transpose(x2[:, j:j + 1],
                                    nkv_row[:, j * 128:(j + 1) * 128],
                                    id1_sb)
            nkvT = nrm_p.tile([128, 4], f32, tag="nkvT")
            nc.vector.tensor_copy(nkvT, x2[:, 0:4])
            return nq_bc, nkv_bc, nkvT

        def b_units(t, nq_bc, nkv_bc, nkvT):
            """list of closures, each one PE chain of phase B for tile t"""
            ts = slice(t * TW, (t + 1) * TW)
            x_sb = x8_tiles[t]
            xb_sb = xb_tiles[t]
            units = []
            holder = {}
            qall_sb = qall_p.tile([128, 6, TW], fp8, tag="qall")

            def qc_unit(h):
                def f():
                    mm = psAB.tile([128, TW], f32, tag="m")
                    dr_chain(mm, wfq_sb, h * 128, x_sb, 8)
                    nc.vector.tensor_mul(qall_sb[:, h, :], mm, nq_bc)
                return f

            def qprep_unit():
                def f():
                    cosn = rope_p.tile([128, TW], f32, tag="cosn")
                    sinn = rope_p.tile([128, TW], f32, tag="sinn")
                    nc.vector.tensor_mul(cosn, cos_sb[:, ts], nq_bc)
                    nc.vector.tensor_mul(sinn, sin_sb[:, ts], nq_bc)
                    holder["cosn"] = cosn
                    holder["sinn"] = sinn
                return f

            def qr_unit(j):
                def f():
                    qr_ps = psAB.tile([128, TW], f32, tag="m")
                    dr_chain(qr_ps, wfq_sb, 512 + j * 128, x_sb, 8)
                    rot_ps = psAB.tile([128, TW], f32, tag="m")
                    dr_chain(rot_ps, wfq_sb, 768 + j * 128, x_sb, 8)
                    t1 = rope_p.tile([128, TW], f32, tag="t1")
                    t2 = rope_p.tile([128, TW], f32, tag="t2")
                    nc.vector.tensor_mul(t1, qr_ps, holder["cosn"])
                    nc.vector.tensor_mul(t2, rot_ps, holder["sinn"])
                    nc.gpsimd.tensor_add(qall_sb[:, 4 + j, :], t1, t2)
                return f

            def k_unit(h):
                def f():
                    mm = psAB.tile([128, TW], f32, tag="m")
                    dr_chain(mm, wfk_sb, h * 128, x_sb, 8)
                    nc.vector.tensor_mul(kall_sb[:, h, ts], mm, nkv_bc)
                return f

            def v_unit(tc4):
                def f():
                    mm = psAB.tile([128, TW], f32, tag="m")
                    for kt in range(16):
                        nc.tensor.matmul(
                            mm,
                            lhsT=xb_sb[:, kt, tc4 * 128:(tc4 + 1) * 128],
                            rhs=wfv_sb[:, kt, :],
                            start=(kt == 0), stop=(kt == 15))
                    nc.vector.tensor_scalar(
                        v_sb[:, t * 4 + tc4, :, :].rearrange(
                            "p h d -> p (h d)"),
                        mm, nkvT[:, tc4:tc4 + 1], V_C,
                        op0=ALU.mult, op1=ALU.mult)
                return f

            units = [qc_unit(h) for h in range(HL)]
            units.append(qprep_unit())
            units += [qr_unit(j) for j in range(2)]
            units += [k_unit(h) for h in range(HL)]
            units += [v_unit(tc4) for tc4 in range(4)]
            return units, qall_sb

        qall_tiles = {}

        def emit_C(tq, fillers):
            """attention for query tile tq with PE filler units woven in"""
            ts = slice(tq * TW, (tq + 1) * TW)
            qall_sb = qall_tiles[tq]
            nkt = 4 * (tq + 1)
            PIPE = 2
            fi = 0
            for h in range(HL):
                qsl = 4 + h // 2 - h
                ksl = 4 + h % 2 - h
                ao_ps = psO.tile([128, TW], f32, tag="o")
                rs_ps = psR.tile([128, TW], f32, tag="r")
                pts = {}
                for kt in range(nkt + PIPE):
                    if kt < nkt:
                        s_ps = psS.tile([128, TW], f32, tag="s")
                        nc.tensor.matmul(
                            s_ps,
                            lhsT=kall_sb[:, h::ksl, kt * 128:(kt + 1) * 128],
                            rhs=qall_sb[:, h::qsl, :],
                            start=True, stop=True, perf_mode=DRM)
                        pt = pt_p.tile([128, TW], bf16, tag="pt")
                        nc.scalar.activation(pt, s_ps, func=AF.Exp,
                                             scale=EXP_SCALE)
                        j = kt - 4 * tq
                        if j >= 0:
                            nc.vector.tensor_mul(pt, pt, mask01[:, j, :])
                        pts[kt] = pt
                    if kt % 3 == 2 and fi < len(fillers):
                        fillers[fi]()
                        fi += 1
                    kd = kt - PIPE
                    if kd >= 0:
                        pt = pts.pop(kd)
                        nc.tensor.matmul(
                            ao_ps, lhsT=v_sb[:, kd, h, :], rhs=pt,
                            start=(kd == 0), stop=(kd == nkt - 1))
                        nc.tensor.matmul(
                            rs_ps, lhsT=onesb, rhs=pt,
                            start=(kd == 0), stop=(kd == nkt - 1))
                rbc = rbc_p.tile([128, TW], f32, tag="rbc")
                nc.vector.reciprocal(rbc, rs_ps)
                nc.vector.tensor_mul(ao_sb[:, h, ts], ao_ps, rbc)
            for f in fillers[fi:]:
                f()

        def d_units(tq):
            ts = slice(tq * TW, (tq + 1) * TW)

            def unit(dc):
                def f():
                    mm = psAB.tile([128, TW], f32, tag="m")
                    for kt4 in range(4):
                        nc.tensor.matmul(
                            mm, lhsT=wo_sb[:, kt4, dc * 128:(dc + 1) * 128],
                            rhs=ao_sb[:, kt4, ts],
                            start=(kt4 == 0), stop=(kt4 == 3))
                    o_sb = od_p.tile([128, TW], f32, tag="od")
                    if dc % 2 == 0:
                        nc.scalar.copy(o_sb, mm)
                    else:
                        nc.vector.tensor_copy(o_sb, mm)
                    nc.gpsimd.dma_start(
                        out=out_d[dc * 128:(dc + 1) * 128, ts], in_=o_sb)
                return f
            return [unit(dc) for dc in range(16)]

        nc.sync.dma_start(out=wdq_sb[:, :, :, 0:512], in_=wdq_d[:, :, :, 0:512])
        load_x(0, xb_too=False)
        prologue()
        load_x(0)
        load_x(1)
        pend_d = []
        for t in range(NT):
            load_x(t + 1)
            nq_bc, nkv_bc, nkvT = emit_A(t, pend_d)
            units, qall_sb = b_units(t, nq_bc, nkv_bc, nkvT)
            qall_tiles[t] = qall_sb
            if t == 0:
                for f in units:
                    f()
            else:
                emit_C(t - 1, units)
                pend_d = d_units(t - 1)
        emit_C(NT - 1, pend_d)
        for f in d_units(NT - 1):
            f()

    nc.finalize()
    return nc


def _rope_tables():
    inv_freq = (1.0 / (ROPE_BASE ** (np.arange(0, DR, 2, dtype=np.float64)
                                     / DR)))
    tt = np.arange(T, dtype=np.float64)
    freqs = np.outer(tt, inv_freq)
    emb = np.concatenate([freqs, freqs], axis=-1)
    cos = np.cos(emb).T
    sin = np.sin(emb).T
    cos128 = np.ascontiguousarray(np.concatenate([cos, cos], 0))
    sin128 = np.ascontiguousarray(np.concatenate([sin, sin], 0))
    return cos128, sin128


def _pair_rows(w):
    """(K, N) -> (128, K//256, 2, N) with row index = pr*256 + two*128 + p"""
    K, N = w.shape
    return np.ascontiguousarray(
        w.reshape(K // 256, 2, 128, N).transpose(2, 0, 1, 3))


def _rows(w):
    """(K, N) -> (128, K//128, N) with row index = c*128 + p"""
    K, N = w.shape
    return np.ascontiguousarray(w.reshape(K // 128, 128, N).transpose(1, 0, 2))


def _rot_cols(w, dr):
    K, N = w.shape
    wh = w.reshape(K, N // dr, dr)
    lo, hi = wh[:, :, :dr // 2], wh[:, :, dr // 2:]
    return np.concatenate([-hi, lo], axis=2).reshape(K, N)


def _to_fp8(a):
    import ml_dtypes
    return np.ascontiguousarray(a).astype(ml_dtypes.float8_e4m3)


def _to_bf16(a):
    import ml_dtypes
    return np.ascontiguousarray(a).astype(ml_dtypes.bfloat16)


def kernel(x, W_DQ, W_UQ, W_QR, W_DKV, W_UK, W_UV, W_KR, W_O,
           q_norm_w, kv_norm_w):
    global LAST_EXEC_NS
    from concourse.bass_utils import run_bass_kernel_spmd

    x = np.asarray(x, dtype=np.float64)
    W_DQ = np.asarray(W_DQ, np.float64)
    W_UQ = np.asarray(W_UQ, np.float64)
    W_QR = np.asarray(W_QR, np.float64)
    W_DKV = np.asarray(W_DKV, np.float64)
    W_UK = np.asarray(W_UK, np.float64)
    W_UV = np.asarray(W_UV, np.float64)
    W_KR = np.asarray(W_KR, np.float64)
    W_O = np.asarray(W_O, np.float64)
    q_norm_w = np.asarray(q_norm_w, np.float64)
    kv_norm_w = np.asarray(kv_norm_w, np.float64)

    # fold norm weights into up-projections (host, f64)
    wuq_f = W_UQ * q_norm_w[:, None]
    wqr_f = W_QR * q_norm_w[:, None]
    wuk_f = W_UK * kv_norm_w[:, None]
    wuv_f = W_UV * kv_norm_w[:, None]

    cos128, sin128 = _rope_tables()
    cos_b = _to_bf16(cos128)
    sin_b = _to_bf16(sin128)

    wuq_h = wuq_f.reshape(DQ, H, DH)
    wqr_h = wqr_f.reshape(DQ, H, DR)
    wuk_h = wuk_f.reshape(DKV, H, DH)
    wuv_h = wuv_f.reshape(DKV, H, DH)
    wo_h = W_O.reshape(H, DH, D)

    wdq_p = _to_fp8(_pair_rows(W_DQ * SA))
    wkv_p = _to_fp8(_pair_rows(W_DKV * SA))
    wkr_cat = np.concatenate(
        [W_KR * SKR, _rot_cols(W_KR.reshape(D, DR), DR) * SKR], axis=1)

    in_maps = []
    for ci in range(NCORES):
        b, hg = divmod(ci, H // HL)
        hsl = slice(hg * HL, (hg + 1) * HL)
        wuq_s = wuq_h[:, hsl].reshape(DQ, HL * DH)
        wqr_s = wqr_h[:, hsl].reshape(DQ, HL * DR)
        wfq = np.concatenate(
            [W_DQ @ wuq_s, W_DQ @ wqr_s, W_DQ @ _rot_cols(wqr_s, DR)],
            axis=1) * SFQ                                    # (D, 1024)
        wuk_s = wuk_h[:, hsl].reshape(DKV, HL * DH)
        wfk = np.concatenate(
            [W_DKV @ wuk_s * SFK, wkr_cat], axis=1)          # (D, 640)
        wfv = W_DKV @ wuv_h[:, hsl].reshape(DKV, HL * DH)    # (D, 512)
        in_maps.append({
            "x8": _to_fp8(_pair_rows(x[b].T)),
            "xb": _to_bf16(_rows(x[b].T)),
            "wdq": wdq_p,
            "wkv": wkv_p,
            "wfq": _to_fp8(_pair_rows(wfq)),
            "wfk": _to_fp8(_pair_rows(wfk)),
            "wfv": _to_bf16(_rows(wfv)),
            "wo": _to_bf16(_rows(wo_h[hsl].reshape(HL * DH, D))),
            "costab": cos_b,
            "sintab": sin_b,
        })

    if "nc" not in _CACHE:
        _CACHE["nc"] = _build()
    nc = _CACHE["nc"]

    import os as _os
    _trace = _os.environ.get("MLA_TRACE") == "1"
    res = run_bass_kernel_spmd(
        nc, in_maps, core_ids=list(range(NCORES)), trace=_trace)
    LAST_EXEC_NS = res.exec_time_ns
    outs = [res.results[ci]["final_t"] for ci in range(NCORES)]

    out = np.zeros((B, T, D), np.float32)
    for ci in range(NCORES):
        b = ci // (H // HL)
        out[b] += outs[ci].T
    return out


# revision 5
# speedup vs baseline: 1.0207x; 1.0171x over previous
"""Multi-Head Latent Attention (MLA) forward on 8 Trainium2 NeuronCores, v3.

Problem shapes (hardcoded, self-contained):
  B=2, T=2048, D=2048, H=16, DH=128, DKV=512, DQ=1024, DR=64, fp32 I/O.

Sharding: core ci = b*4 + hg; each core owns one batch element and 4 heads.
Up-projection weights sharded over heads; W_O input-dim sharded, each core
emits a partial (D,T) output summed on host.

Mixed-precision strategy (validated against an element-exact numpy model):
  * Q/K score inputs tolerate fp8-e4m3 noise (softmax diversifies it), so
    the entire Q and K paths run as HOST-FUSED single matmuls
    x @ (W_DQ@W_UQ), x @ (W_DKV@W_UK), x @ W_KR in fp8 DoubleRow mode
    (two 128-row k-tiles per instruction at 0.5 cycles/output element).
    Fusing means each path takes ONE fp8 dot-noise instead of two.
  * The V path, attention probabilities, attention output, and the final
    W_O projection stay bf16 end-to-end: peaked causal attention passes
    V-path noise straight to the output max-error metric.
  * RMSNorm denominators come from dedicated fp8 DoubleRow chains
    (x@W_DQ, x@W_DKV) whose only consumers are sums of squares; the
    rsqrt is a single ACT Rsqrt with all fp8/host scales folded into its
    scale/bias constants.  Norm multipliers are folded into the psum->sbuf
    quantize ops (never a separate pass).
  * Rotate-half is produced by matmul against host-permuted weight columns.
  * Causal masking multiplies exp outputs by a 0/1 bf16 mask on Pool,
    off the PSUM critical path.  Row-sums use an all-ones bf16 lhsT whose
    output lands pre-broadcast across all 128 partitions.
"""

import math

import numpy as np

B, T, D = 2, 2048, 2048
H, DH = 16, 128
DKV, DQ, DR = 512, 1024, 64
ROPE_BASE = 500000.0
EPS = 1e-6
SCALE = 1.0 / math.sqrt(DH + DR)

HL = 4            # heads per core
NCORES = 8
TW = 512          # token tile width
NT = T // TW      # 4 token tiles

# fp8 scale plan (powers of two)
SA = 16.0         # W_DQ / W_DKV norm-chain quantization scale
SFQ = 32.0        # fused Q weight scale
SFK = 32.0        # fused K weight scale
SKR = 16.0        # W_KR scale (columns inside the fused K tensor)
SQ = 2.0          # q/k value scale in fp8
EXP_SCALE = SCALE / (SQ * SQ)
LNA_Q = (SFQ / SQ) ** 2 / (SA * SA * DQ)
LNB_Q = (SFQ / SQ) ** 2 * EPS
LNA_K = (SFK / SQ) ** 2 / (SA * SA * DKV)
LNB_K = (SFK / SQ) ** 2 * EPS
KR_C = SQ / SKR
V_C = SFK / SQ    # v multiplier: nkv_row * V_C = true rsqrt(ms+eps)

_CACHE: dict = {}
LAST_EXEC_NS = None


def _build():
    from contextlib import ExitStack

    import concourse.mybir as mybir
    import concourse.tile as tile
    from concourse.bacc import Bacc

    f32 = mybir.dt.float32
    bf16 = mybir.dt.bfloat16
    fp8 = mybir.dt.float8e4
    AF = mybir.ActivationFunctionType
    DRM = mybir.MatmulPerfMode.DoubleRow
    ALU = mybir.AluOpType

    nc = Bacc("TRN2")

    # fp8 tensors arrive pre-paired: [128, npair, 2, N], row = pr*256+two*128+p
    x8_d = nc.dram_tensor("x8", (128, 8, 2, T), fp8, kind="ExternalInput")
    xb_d = nc.dram_tensor("xb", (128, 16, T), bf16, kind="ExternalInput")
    wdq_d = nc.dram_tensor("wdq", (128, 8, 2, DQ), fp8, kind="ExternalInput")
    wkv_d = nc.dram_tensor("wkv", (128, 8, 2, DKV), fp8, kind="ExternalInput")
    wfq_d = nc.dram_tensor("wfq", (128, 8, 2, 1024), fp8, kind="ExternalInput")
    wfk_d = nc.dram_tensor("wfk", (128, 8, 2, 640), fp8, kind="ExternalInput")
    wfv_d = nc.dram_tensor("wfv", (128, 16, 512), bf16, kind="ExternalInput")
    wo_d = nc.dram_tensor("wo", (128, 4, D), bf16, kind="ExternalInput")
    cos_d = nc.dram_tensor("costab", (128, T), bf16, kind="ExternalInput")
    sin_d = nc.dram_tensor("sintab", (128, T), bf16, kind="ExternalInput")
    out_d = nc.dram_tensor("final_t", (D, T), bf16, kind="ExternalOutput")

    with tile.TileContext(nc) as tc, ExitStack() as ctx:
        pers = ctx.enter_context(tc.tile_pool(name="pers", bufs=1))

        # ---------- persistent weights (DMA in use order) ----------
        wdq_sb = pers.tile([128, 8, 2, DQ], fp8, tag="wdq")
        wkv_sb = pers.tile([128, 8, 2, DKV], fp8, tag="wkv")
        wfq_sb = pers.tile([128, 8, 2, 1024], fp8, tag="wfq")
        wfk_sb = pers.tile([128, 8, 2, 640], fp8, tag="wfk")
        wfv_sb = pers.tile([128, 16, 512], bf16, tag="wfv")
        wo_sb = pers.tile([128, 4, D], bf16, tag="wo")
        cos_sb = pers.tile([128, T], bf16, tag="cos")
        sin_sb = pers.tile([128, T], bf16, tag="sin")

        # ---------- persistent activations ----------
        # kall slots: 0-3 content per head, 4 [kr;0], 5 [0;kr]   (fp8, SQ*khat)
        kall_sb = pers.tile([128, 6, T], fp8, tag="kall")
        # v token-major bf16: [128 tok, kt 16, head, dh]
        v_sb = pers.tile([128, 16, HL, DH], bf16, tag="v")
        # attention out feature-major bf16: [128 dh, head, T]
        ao_sb = pers.tile([128, HL, T], bf16, tag="ao")

        # ---------- constants ----------
        onesb = pers.tile([128, 128], bf16, tag="onesb")
        twosb = pers.tile([128, 1], bf16, tag="twosb")
        lnbq_sb = pers.tile([1, 1], f32, tag="lnbq")
        lnbk_sb = pers.tile([1, 1], f32, tag="lnbk")
        id1_sb = pers.tile([1, 1], f32, tag="id1")
        mask01 = pers.tile([128, 4, TW], bf16, tag="mask01")

        def prologue():
            nc.sync.dma_start(out=wdq_sb[:, :, :, 512:1024], in_=wdq_d[:, :, :, 512:1024])
            nc.sync.dma_start(out=wfq_sb, in_=wfq_d[:, :, :, :])
            nc.sync.dma_start(out=wkv_sb, in_=wkv_d[:, :, :, :])
            nc.sync.dma_start(out=wfk_sb, in_=wfk_d[:, :, :, :])
            nc.sync.dma_start(out=wfv_sb, in_=wfv_d[:, :, :])
            nc.sync.dma_start(out=cos_sb, in_=cos_d[:, :])
            nc.sync.dma_start(out=sin_sb, in_=sin_d[:, :])
            nc.sync.dma_start(out=wo_sb, in_=wo_d[:, :, :])
            nc.gpsimd.memset(kall_sb[:, 4:6, :], 0.0)
            nc.gpsimd.memset(onesb, 1.0)
            nc.gpsimd.memset(twosb, 2.0)
            nc.vector.memset(lnbq_sb, LNB_Q)
            nc.vector.memset(lnbk_sb, LNB_K)
            nc.vector.memset(id1_sb, 1.0)
            nc.gpsimd.memset(mask01, 1.0)
            for j in range(4):
                nc.gpsimd.affine_select(
                    out=mask01[:, j, :], in_=mask01[:, j, :],
                    compare_op=ALU.is_ge, fill=0.0,
                    base=-128 * j, pattern=[[1, TW]], channel_multiplier=-1)


        # ---------- pools ----------
        x8in = ctx.enter_context(tc.tile_pool(name="x8in", bufs=2))
        xbin = ctx.enter_context(tc.tile_pool(name="xbin", bufs=1))
        qall_p = ctx.enter_context(tc.tile_pool(name="qall", bufs=2))
        sq_p = ctx.enter_context(tc.tile_pool(name="sq", bufs=1))
        nrm_p = ctx.enter_context(tc.tile_pool(name="nrm", bufs=1))
        rope_p = ctx.enter_context(tc.tile_pool(name="rope", bufs=1))
        pt_p = ctx.enter_context(tc.tile_pool(name="pt", bufs=4))
        rbc_p = ctx.enter_context(tc.tile_pool(name="rbc", bufs=1))
        od_p = ctx.enter_context(tc.tile_pool(name="od", bufs=6))

        psAB = ctx.enter_context(tc.tile_pool(name="psAB", bufs=4, space="PSUM"))
        psS = ctx.enter_context(tc.tile_pool(name="psS", bufs=2, space="PSUM"))
        psO = ctx.enter_context(tc.tile_pool(name="psO", bufs=1, space="PSUM"))
        psR = ctx.enter_context(tc.tile_pool(name="psR", bufs=1, space="PSUM"))

        def dr_chain(out_ps, w_sb, col0, x_sb, npair):
            for pr in range(npair):
                nc.tensor.matmul(
                    out_ps,
                    lhsT=w_sb[:, pr, :, col0:col0 + 128],
                    rhs=x_sb[:, pr, :, :],
                    start=(pr == 0), stop=(pr == npair - 1),
                    perf_mode=DRM)

        # ---------- main loop with cross-phase interleaving ----------
        x8_tiles = {}
        xb_tiles = {}

        def load_x(t, xb_too=True):
            ts = slice(t * TW, (t + 1) * TW)
            if t < NT and t not in x8_tiles:
                xs = x8in.tile([128, 8, 2, TW], fp8, tag="x8")
                nc.sync.dma_start(out=xs, in_=x8_d[:, :, :, ts])
                x8_tiles[t] = xs
            if xb_too and t < NT and t not in xb_tiles:
                xbs = xbin.tile([128, 16, TW], bf16, tag="xb")
                nc.sync.dma_start(out=xbs, in_=xb_d[:, :, ts])
                xb_tiles[t] = xbs

        def emit_A(t, dfill):
            """norm chains + kr; returns (nq_bc, nkv_bc, nkvT) tiles"""
            ts = slice(t * TW, (t + 1) * TW)
            x_sb = x8_tiles[t]
            di = [0]

            def dpop():
                if di[0] < len(dfill):
                    dfill[di[0]]()
                    di[0] += 1
            sum_q = psS.tile([128, TW], f32, tag="s")
            sum_k = psS.tile([128, TW], f32, tag="s")
            sq_q = []
            for mc in range(12):
                mm = psAB.tile([128, TW], f32, tag="m")
                if mc < 8:
                    dr_chain(mm, wdq_sb, mc * 128, x_sb, 8)
                else:
                    dr_chain(mm, wkv_sb, (mc - 8) * 128, x_sb, 8)
                sq = sq_p.tile([128, TW], bf16, tag=f"sq{mc % 2}")
                nc.scalar.square(sq, mm)
                sq_q.append(sq)
                dpop()
                if mc >= 1:
                    k = mc - 1
                    sqd = sq_q[k]
                    if k < 8:
                        nc.tensor.matmul(sum_q[0:1, :], lhsT=onesb[:, 0:1],
                                         rhs=sqd, start=(k == 0),
                                         stop=(k == 7))
                    else:
                        nc.tensor.matmul(sum_k[0:1, :],
                                         lhsT=twosb[:, 0:1], rhs=sqd,
                                         start=(k == 8), stop=False)
            sqd = sq_q[11]
            nc.tensor.matmul(sum_k[0:1, :], lhsT=twosb[:, 0:1],
                             rhs=sqd, start=False, stop=True)
            # kr chain (no norm dependency)
            mm = psAB.tile([128, TW], f32, tag="m")
            dr_chain(mm, wfk_sb, 512, x_sb, 8)
            krt = rope_p.tile([128, TW], bf16, tag="krt")
            nc.scalar.copy(krt, mm)
            t1 = rope_p.tile([128, TW], f32, tag="t1")
            t2 = rope_p.tile([128, TW], f32, tag="t2")
            nc.vector.scalar_tensor_tensor(
                t1[0:64, :], krt[0:64, :], KR_C, cos_sb[0:64, ts],
                op0=ALU.mult, op1=ALU.mult)
            nc.vector.scalar_tensor_tensor(
                t2[0:64, :], krt[64:128, :], KR_C, sin_sb[64:128, ts],
                op0=ALU.mult, op1=ALU.mult)
            nc.gpsimd.tensor_add(kall_sb[0:64, 4, ts], t1[0:64, :],
                                 t2[0:64, :])
            nc.vector.tensor_copy(kall_sb[64:128, 5, ts],
                                  kall_sb[0:64, 4, ts])
            while di[0] < len(dfill):
                dfill[di[0]]()
                di[0] += 1
            # norms: Ln,Ln then Exp,Exp keeps table switches minimal
            n2_row = nrm_p.tile([1, 2 * TW], f32, tag="n2")
            nq_row = n2_row[:, 0:TW]
            nkv_row = n2_row[:, TW:2 * TW]
            nc.scalar.activation(nq_row, sum_q[0:1, :], func=AF.Ln,
                                 scale=LNA_Q, bias=lnbq_sb)
            nc.scalar.activation(nkv_row, sum_k[0:1, :], func=AF.Ln,
                                 scale=LNA_Q, bias=lnbq_sb)
            nc.scalar.activation(n2_row, n2_row, func=AF.Exp, scale=-0.5)
            nq_bc = nrm_p.tile([128, TW], f32, tag="nqbc")
            nc.gpsimd.partition_broadcast(nq_bc, nq_row)
            nkv_bc = nrm_p.tile([128, TW], f32, tag="nkvbc")
            nc.gpsimd.partition_broadcast(nkv_bc, nkv_row)
            x2 = psO.tile([128, TW], f32, tag="o")
            for j in range(4):
                nc.tensor.transpose(x2[:, j:j + 1],
                                    nkv_row[:, j * 128:(j + 1) * 128],
                                    id1_sb)
            nkvT = nrm_p.tile([128, 4], f32, tag="nkvT")
            nc.vector.tensor_copy(nkvT, x2[:, 0:4])
            return nq_bc, nkv_bc, nkvT

        def b_units(t, nq_bc, nkv_bc, nkvT):
            """list of closures, each one PE chain of phase B for tile t"""
            ts = slice(t * TW, (t + 1) * TW)
            x_sb = x8_tiles[t]
            xb_sb = xb_tiles[t]
            units = []
            holder = {}
            qall_sb = qall_p.tile([128, 6, TW], fp8, tag="qall")

            def qc_unit(h):
                def f():
                    mm = psAB.tile([128, TW], f32, tag="m")
                    dr_chain(mm, wfq_sb, h * 128, x_sb, 8)
                    nc.vector.tensor_mul(qall_sb[:, h, :], mm, nq_bc)
                return f

            def qprep_unit():
                def f():
                    cosn = rope_p.tile([128, TW], f32, tag="cosn")
                    sinn = rope_p.tile([128, TW], f32, tag="sinn")
                    nc.vector.tensor_mul(cosn, cos_sb[:, ts], nq_bc)
                    nc.vector.tensor_mul(sinn, sin_sb[:, ts], nq_bc)
                    holder["cosn"] = cosn
                    holder["sinn"] = sinn
                return f

            def qr_unit(j):
                def f():
                    qr_ps = psAB.tile([128, TW], f32, tag="m")
                    dr_chain(qr_ps, wfq_sb, 512 + j * 128, x_sb, 8)
                    rot_ps = psAB.tile([128, TW], f32, tag="m")
                    dr_chain(rot_ps, wfq_sb, 768 + j * 128, x_sb, 8)
                    t1 = rope_p.tile([128, TW], f32, tag="t1")
                    t2 = rope_p.tile([128, TW], f32, tag="t2")
                    nc.vector.tensor_mul(t1, qr_ps, holder["cosn"])
                    nc.vector.tensor_mul(t2, rot_ps, holder["sinn"])
                    nc.gpsimd.tensor_add(qall_sb[:, 4 + j, :], t1, t2)
                return f

            def k_unit(h):
                def f():
                    mm = psAB.tile([128, TW], f32, tag="m")
                    dr_chain(mm, wfk_sb, h * 128, x_sb, 8)
                    nc.vector.tensor_mul(kall_sb[:, h, ts], mm, nkv_bc)
                return f

            def v_unit(tc4):
                def f():
                    mm = psAB.tile([128, TW], f32, tag="m")
                    for kt in range(16):
                        nc.tensor.matmul(
                            mm,
                            lhsT=xb_sb[:, kt, tc4 * 128:(tc4 + 1) * 128],
                            rhs=wfv_sb[:, kt, :],
                            start=(kt == 0), stop=(kt == 15))
                    nc.vector.tensor_scalar(
                        v_sb[:, t * 4 + tc4, :, :].rearrange(
                            "p h d -> p (h d)"),
                        mm, nkvT[:, tc4:tc4 + 1], V_C,
                        op0=ALU.mult, op1=ALU.mult)
                return f

            units = [qc_unit(h) for h in range(HL)]
            units.append(qprep_unit())
            units += [qr_unit(j) for j in range(2)]
            units += [k_unit(h) for h in range(HL)]
            units += [v_unit(tc4) for tc4 in range(4)]
            return units, qall_sb

        qall_tiles = {}

        def emit_C(tq, fillers):
            """attention for query tile tq with PE filler units woven in"""
            ts = slice(tq * TW, (tq + 1) * TW)
            qall_sb = qall_tiles[tq]
            nkt = 4 * (tq + 1)
            PIPE = 2
            fi = 0
            for h in range(HL):
                qsl = 4 + h // 2 - h
                ksl = 4 + h % 2 - h
                ao_ps = psO.tile([128, TW], f32, tag="o")
                rs_ps = psR.tile([128, TW], f32, tag="r")
                pts = {}
                for kt in range(nkt + PIPE):
                    if kt < nkt:
                        s_ps = psS.tile([128, TW], f32, tag="s")
                        nc.tensor.matmul(
                            s_ps,
                            lhsT=kall_sb[:, h::ksl, kt * 128:(kt + 1) * 128],
                            rhs=qall_sb[:, h::qsl, :],
                            start=True, stop=True, perf_mode=DRM)
                        pt = pt_p.tile([128, TW], bf16, tag="pt")
                        nc.scalar.activation(pt, s_ps, func=AF.Exp,
                                             scale=EXP_SCALE)
                        j = kt - 4 * tq
                        if j >= 0:
                            nc.vector.tensor_mul(pt, pt, mask01[:, j, :])
                        pts[kt] = pt
                    if kt % 3 == 2 and fi < len(fillers):
                        fillers[fi]()
                        fi += 1
                    kd = kt - PIPE
                    if kd >= 0:
                        pt = pts.pop(kd)
                        nc.tensor.matmul(
                            ao_ps, lhsT=v_sb[:, kd, h, :], rhs=pt,
                            start=(kd == 0), stop=(kd == nkt - 1))
                        nc.tensor.matmul(
                            rs_ps, lhsT=onesb, rhs=pt,
                            start=(kd == 0), stop=(kd == nkt - 1))
                rbc = rbc_p.tile([128, TW], f32, tag="rbc")
                nc.vector.reciprocal(rbc, rs_ps)
                nc.vector.tensor_mul(ao_sb[:, h, ts], ao_ps, rbc)
            for f in fillers[fi:]:
                f()

        def d_units(tq):
            ts = slice(tq * TW, (tq + 1) * TW)

            def unit(dc):
                def f():
                    mm = psAB.tile([128, TW], f32, tag="m")
                    for kt4 in range(4):
                        nc.tensor.matmul(
                            mm, lhsT=wo_sb[:, kt4, dc * 128:(dc + 1) * 128],
                            rhs=ao_sb[:, kt4, ts],
                            start=(kt4 == 0), stop=(kt4 == 3))
                    o_sb = od_p.tile([128, TW], bf16, tag="od")
                    if dc % 2 == 0:
                        nc.scalar.copy(o_sb, mm)
                    else:
                        nc.vector.tensor_copy(o_sb, mm)
                    nc.gpsimd.dma_start(
                        out=out_d[dc * 128:(dc + 1) * 128, ts], in_=o_sb)
                return f
            return [unit(dc) for dc in range(16)]

        nc.sync.dma_start(out=wdq_sb[:, :, :, 0:512], in_=wdq_d[:, :, :, 0:512])
        load_x(0, xb_too=False)
        prologue()
        load_x(0)
        load_x(1)
        pend_d = []
        for t in range(NT):
            load_x(t + 1)
            nq_bc, nkv_bc, nkvT = emit_A(t, pend_d)
            units, qall_sb = b_units(t, nq_bc, nkv_bc, nkvT)
            qall_tiles[t] = qall_sb
            if t == 0:
                for f in units:
                    f()
            else:
                emit_C(t - 1, units)
                pend_d = d_units(t - 1)
        emit_C(NT - 1, pend_d)
        for f in d_units(NT - 1):
            f()

    nc.finalize()
    return nc


def _rope_tables():
    inv_freq = (1.0 / (ROPE_BASE ** (np.arange(0, DR, 2, dtype=np.float64)
                                     / DR)))
    tt = np.arange(T, dtype=np.float64)
    freqs = np.outer(tt, inv_freq)
    emb = np.concatenate([freqs, freqs], axis=-1)
    cos = np.cos(emb).T
    sin = np.sin(emb).T
    cos128 = np.ascontiguousarray(np.concatenate([cos, cos], 0))
    sin128 = np.ascontiguousarray(np.concatenate([sin, sin], 0))
    return cos128, sin128


def _pair_rows(w):
    """(K, N) -> (128, K//256, 2, N) with row index = pr*256 + two*128 + p"""
    K, N = w.shape
    return np.ascontiguousarray(
        w.reshape(K // 256, 2, 128, N).transpose(2, 0, 1, 3))


def _rows(w):
    """(K, N) -> (128, K//128, N) with row index = c*128 + p"""
    K, N = w.shape
    return np.ascontiguousarray(w.reshape(K // 128, 128, N).transpose(1, 0, 2))


def _rot_cols(w, dr):
    K, N = w.shape
    wh = w.reshape(K, N // dr, dr)
    lo, hi = wh[:, :, :dr // 2], wh[:, :, dr // 2:]
    return np.concatenate([-hi, lo], axis=2).reshape(K, N)


def _to_fp8(a):
    import ml_dtypes
    return np.ascontiguousarray(a).astype(ml_dtypes.float8_e4m3)


def _to_bf16(a):
    import ml_dtypes
    return np.ascontiguousarray(a).astype(ml_dtypes.bfloat16)


def kernel(x, W_DQ, W_UQ, W_QR, W_DKV, W_UK, W_UV, W_KR, W_O,
           q_norm_w, kv_norm_w):
    global LAST_EXEC_NS
    from concourse.bass_utils import run_bass_kernel_spmd

    x = np.asarray(x, dtype=np.float64)
    W_DQ = np.asarray(W_DQ, np.float64)
    W_UQ = np.asarray(W_UQ, np.float64)
    W_QR = np.asarray(W_QR, np.float64)
    W_DKV = np.asarray(W_DKV, np.float64)
    W_UK = np.asarray(W_UK, np.float64)
    W_UV = np.asarray(W_UV, np.float64)
    W_KR = np.asarray(W_KR, np.float64)
    W_O = np.asarray(W_O, np.float64)
    q_norm_w = np.asarray(q_norm_w, np.float64)
    kv_norm_w = np.asarray(kv_norm_w, np.float64)

    # fold norm weights into up-projections (host, f64)
    wuq_f = W_UQ * q_norm_w[:, None]
    wqr_f = W_QR * q_norm_w[:, None]
    wuk_f = W_UK * kv_norm_w[:, None]
    wuv_f = W_UV * kv_norm_w[:, None]

    cos128, sin128 = _rope_tables()
    cos_b = _to_bf16(cos128)
    sin_b = _to_bf16(sin128)

    wuq_h = wuq_f.reshape(DQ, H, DH)
    wqr_h = wqr_f.reshape(DQ, H, DR)
    wuk_h = wuk_f.reshape(DKV, H, DH)
    wuv_h = wuv_f.reshape(DKV, H, DH)
    wo_h = W_O.reshape(H, DH, D)

    wdq_p = _to_fp8(_pair_rows(W_DQ * SA))
    wkv_p = _to_fp8(_pair_rows(W_DKV * SA))
    wkr_cat = np.concatenate(
        [W_KR * SKR, _rot_cols(W_KR.reshape(D, DR), DR) * SKR], axis=1)

    in_maps = []
    for ci in range(NCORES):
        b, hg = divmod(ci, H // HL)
        hsl = slice(hg * HL, (hg + 1) * HL)
        wuq_s = wuq_h[:, hsl].reshape(DQ, HL * DH)
        wqr_s = wqr_h[:, hsl].reshape(DQ, HL * DR)
        wfq = np.concatenate(
            [W_DQ @ wuq_s, W_DQ @ wqr_s, W_DQ @ _rot_cols(wqr_s, DR)],
            axis=1) * SFQ                                    # (D, 1024)
        wuk_s = wuk_h[:, hsl].reshape(DKV, HL * DH)
        wfk = np.concatenate(
            [W_DKV @ wuk_s * SFK, wkr_cat], axis=1)          # (D, 640)
        wfv = W_DKV @ wuv_h[:, hsl].reshape(DKV, HL * DH)    # (D, 512)
        in_maps.append({
            "x8": _to_fp8(_pair_rows(x[b].T)),
            "xb": _to_bf16(_rows(x[b].T)),
            "wdq": wdq_p,
            "wkv": wkv_p,
            "wfq": _to_fp8(_pair_rows(wfq)),
            "wfk": _to_fp8(_pair_rows(wfk)),
            "wfv": _to_bf16(_rows(wfv)),
            "wo": _to_bf16(_rows(wo_h[hsl].reshape(HL * DH, D))),
            "costab": cos_b,
            "sintab": sin_b,
        })

    if "nc" not in _CACHE:
        _CACHE["nc"] = _build()
    nc = _CACHE["nc"]

    import os as _os
    _trace = _os.environ.get("MLA_TRACE") == "1"
    res = run_bass_kernel_spmd(
        nc, in_maps, core_ids=list(range(NCORES)), trace=_trace)
    LAST_EXEC_NS = res.exec_time_ns
    outs = [res.results[ci]["final_t"] for ci in range(NCORES)]

    out = np.zeros((B, T, D), np.float32)
    for ci in range(NCORES):
        b = ci // (H // HL)
        out[b] += np.asarray(outs[ci], dtype=np.float32).T
    return out


# revision 6
# speedup vs baseline: 1.0284x; 1.0076x over previous
"""Multi-Head Latent Attention (MLA) forward on 8 Trainium2 NeuronCores, v3.

Problem shapes (hardcoded, self-contained):
  B=2, T=2048, D=2048, H=16, DH=128, DKV=512, DQ=1024, DR=64, fp32 I/O.

Sharding: core ci = b*4 + hg; each core owns one batch element and 4 heads.
Up-projection weights sharded over heads; W_O input-dim sharded, each core
emits a partial (D,T) output summed on host.

Mixed-precision strategy (validated against an element-exact numpy model):
  * Q/K score inputs tolerate fp8-e4m3 noise (softmax diversifies it), so
    the entire Q and K paths run as HOST-FUSED single matmuls
    x @ (W_DQ@W_UQ), x @ (W_DKV@W_UK), x @ W_KR in fp8 DoubleRow mode
    (two 128-row k-tiles per instruction at 0.5 cycles/output element).
    Fusing means each path takes ONE fp8 dot-noise instead of two.
  * The V path, attention probabilities, attention output, and the final
    W_O projection stay bf16 end-to-end: peaked causal attention passes
    V-path noise straight to the output max-error metric.
  * RMSNorm denominators come from dedicated fp8 DoubleRow chains
    (x@W_DQ, x@W_DKV) whose only consumers are sums of squares; the
    rsqrt is a single ACT Rsqrt with all fp8/host scales folded into its
    scale/bias constants.  Norm multipliers are folded into the psum->sbuf
    quantize ops (never a separate pass).
  * Rotate-half is produced by matmul against host-permuted weight columns.
  * Causal masking multiplies exp outputs by a 0/1 bf16 mask on Pool,
    off the PSUM critical path.  Row-sums use an all-ones bf16 lhsT whose
    output lands pre-broadcast across all 128 partitions.
"""

import math

import numpy as np

B, T, D = 2, 2048, 2048
H, DH = 16, 128
DKV, DQ, DR = 512, 1024, 64
ROPE_BASE = 500000.0
EPS = 1e-6
SCALE = 1.0 / math.sqrt(DH + DR)

HL = 4            # heads per core
NCORES = 8
TW = 512          # token tile width
NT = T // TW      # 4 token tiles

# fp8 scale plan (powers of two)
SA = 16.0         # W_DQ / W_DKV norm-chain quantization scale
SFQ = 32.0        # fused Q weight scale
SFK = 32.0        # fused K weight scale
SKR = 16.0        # W_KR scale (columns inside the fused K tensor)
SQ = 2.0          # q/k value scale in fp8
EXP_SCALE = SCALE / (SQ * SQ)
LNA_Q = (SFQ / SQ) ** 2 / (SA * SA * DQ)
LNB_Q = (SFQ / SQ) ** 2 * EPS
LNA_K = (SFK / SQ) ** 2 / (SA * SA * DKV)
LNB_K = (SFK / SQ) ** 2 * EPS
KR_C = SQ / SKR
V_C = SFK / SQ    # v multiplier: nkv_row * V_C = true rsqrt(ms+eps)

_CACHE: dict = {}
LAST_EXEC_NS = None


def _build():
    from contextlib import ExitStack

    import concourse.mybir as mybir
    import concourse.tile as tile
    from concourse.bacc import Bacc

    f32 = mybir.dt.float32
    bf16 = mybir.dt.bfloat16
    fp8 = mybir.dt.float8e4
    AF = mybir.ActivationFunctionType
    DRM = mybir.MatmulPerfMode.DoubleRow
    ALU = mybir.AluOpType

    nc = Bacc("TRN2")

    # fp8 tensors arrive pre-paired: [128, npair, 2, N], row = pr*256+two*128+p
    x8_d = nc.dram_tensor("x8", (128, 8, 2, T), fp8, kind="ExternalInput")
    xb_d = nc.dram_tensor("xb", (128, 16, T), bf16, kind="ExternalInput")
    wdq_d = nc.dram_tensor("wdq", (128, 8, 2, DQ), fp8, kind="ExternalInput")
    wkv_d = nc.dram_tensor("wkv", (128, 8, 2, DKV), fp8, kind="ExternalInput")
    wfq_d = nc.dram_tensor("wfq", (128, 8, 2, 1024), fp8, kind="ExternalInput")
    wfk_d = nc.dram_tensor("wfk", (128, 8, 2, 640), fp8, kind="ExternalInput")
    wfv_d = nc.dram_tensor("wfv", (128, 16, 512), bf16, kind="ExternalInput")
    wo_d = nc.dram_tensor("wo", (128, 4, D), bf16, kind="ExternalInput")
    cos_d = nc.dram_tensor("costab", (128, T), bf16, kind="ExternalInput")
    sin_d = nc.dram_tensor("sintab", (128, T), bf16, kind="ExternalInput")
    out_d = nc.dram_tensor("final_t", (D, T), bf16, kind="ExternalOutput")

    with tile.TileContext(nc) as tc, ExitStack() as ctx:
        pers = ctx.enter_context(tc.tile_pool(name="pers", bufs=1))

        # ---------- persistent weights (DMA in use order) ----------
        wdq_sb = pers.tile([128, 8, 2, DQ], fp8, tag="wdq")
        wkv_sb = pers.tile([128, 8, 2, DKV], fp8, tag="wkv")
        wfq_sb = pers.tile([128, 8, 2, 1024], fp8, tag="wfq")
        wfk_sb = pers.tile([128, 8, 2, 640], fp8, tag="wfk")
        wfv_sb = pers.tile([128, 16, 512], bf16, tag="wfv")
        wo_sb = pers.tile([128, 4, D], bf16, tag="wo")
        cos_sb = pers.tile([128, T], bf16, tag="cos")
        sin_sb = pers.tile([128, T], bf16, tag="sin")

        # ---------- persistent activations ----------
        # kall slots: 0-3 content per head, 4 [kr;0], 5 [0;kr]   (fp8, SQ*khat)
        kall_sb = pers.tile([128, 6, T], fp8, tag="kall")
        # v token-major bf16: [128 tok, kt 16, head, dh]
        v_sb = pers.tile([128, 16, HL, DH], bf16, tag="v")
        # attention out feature-major bf16: [128 dh, head, T]
        ao_sb = pers.tile([128, HL, T], bf16, tag="ao")

        # ---------- constants ----------
        onesb = pers.tile([128, 128], bf16, tag="onesb")
        twosb = pers.tile([128, 1], bf16, tag="twosb")
        lnbq_sb = pers.tile([1, 1], f32, tag="lnbq")
        lnbk_sb = pers.tile([1, 1], f32, tag="lnbk")
        id1_sb = pers.tile([1, 1], f32, tag="id1")
        mask01 = pers.tile([128, 4, TW], bf16, tag="mask01")

        def prologue():
            nc.sync.dma_start(out=wdq_sb[:, :, :, 512:1024], in_=wdq_d[:, :, :, 512:1024])
            nc.sync.dma_start(out=wfq_sb, in_=wfq_d[:, :, :, :])
            nc.sync.dma_start(out=wkv_sb, in_=wkv_d[:, :, :, :])
            nc.sync.dma_start(out=wfk_sb, in_=wfk_d[:, :, :, :])
            nc.sync.dma_start(out=wfv_sb, in_=wfv_d[:, :, :])
            nc.sync.dma_start(out=cos_sb, in_=cos_d[:, :])
            nc.sync.dma_start(out=sin_sb, in_=sin_d[:, :])
            nc.sync.dma_start(out=wo_sb, in_=wo_d[:, :, :])
            nc.gpsimd.memset(kall_sb[:, 4:6, :], 0.0)
            nc.gpsimd.memset(onesb, 1.0)
            nc.gpsimd.memset(twosb, 2.0)
            nc.vector.memset(lnbq_sb, LNB_Q)
            nc.vector.memset(lnbk_sb, LNB_K)
            nc.vector.memset(id1_sb, 1.0)
            nc.gpsimd.memset(mask01, 1.0)
            for j in range(4):
                nc.gpsimd.affine_select(
                    out=mask01[:, j, :], in_=mask01[:, j, :],
                    compare_op=ALU.is_ge, fill=0.0,
                    base=-128 * j, pattern=[[1, TW]], channel_multiplier=-1)


        # ---------- pools ----------
        x8in = ctx.enter_context(tc.tile_pool(name="x8in", bufs=2))
        xbin = ctx.enter_context(tc.tile_pool(name="xbin", bufs=1))
        qall_p = ctx.enter_context(tc.tile_pool(name="qall", bufs=2))
        sq_p = ctx.enter_context(tc.tile_pool(name="sq", bufs=1))
        nrm_p = ctx.enter_context(tc.tile_pool(name="nrm", bufs=1))
        rope_p = ctx.enter_context(tc.tile_pool(name="rope", bufs=1))
        pt_p = ctx.enter_context(tc.tile_pool(name="pt", bufs=4))
        rbc_p = ctx.enter_context(tc.tile_pool(name="rbc", bufs=1))
        od_p = ctx.enter_context(tc.tile_pool(name="od", bufs=6))

        psAB = ctx.enter_context(tc.tile_pool(name="psAB", bufs=4, space="PSUM"))
        psS = ctx.enter_context(tc.tile_pool(name="psS", bufs=2, space="PSUM"))
        psO = ctx.enter_context(tc.tile_pool(name="psO", bufs=1, space="PSUM"))
        psR = ctx.enter_context(tc.tile_pool(name="psR", bufs=1, space="PSUM"))

        def dr_chain(out_ps, w_sb, col0, x_sb, npair):
            for pr in range(npair):
                nc.tensor.matmul(
                    out_ps,
                    lhsT=w_sb[:, pr, :, col0:col0 + 128],
                    rhs=x_sb[:, pr, :, :],
                    start=(pr == 0), stop=(pr == npair - 1),
                    perf_mode=DRM)

        # ---------- main loop with cross-phase interleaving ----------
        x8_tiles = {}
        xb_tiles = {}

        def load_x(t, xb_too=True):
            ts = slice(t * TW, (t + 1) * TW)
            if t < NT and t not in x8_tiles:
                xs = x8in.tile([128, 8, 2, TW], fp8, tag="x8")
                nc.sync.dma_start(out=xs, in_=x8_d[:, :, :, ts])
                x8_tiles[t] = xs
            if xb_too and t < NT and t not in xb_tiles:
                xbs = xbin.tile([128, 16, TW], bf16, tag="xb")
                nc.sync.dma_start(out=xbs, in_=xb_d[:, :, ts])
                xb_tiles[t] = xbs

        def emit_A(t, dfill):
            """norm chains + kr; returns (nq_bc, nkv_bc, nkvT) tiles"""
            ts = slice(t * TW, (t + 1) * TW)
            x_sb = x8_tiles[t]
            di = [0]

            def dpop():
                if di[0] < len(dfill):
                    dfill[di[0]]()
                    di[0] += 1
            sum_q = psS.tile([128, TW], f32, tag="s")
            sum_k = psS.tile([128, TW], f32, tag="s")
            sq_q = []
            for mc in range(12):
                mm = psAB.tile([128, TW], f32, tag="m")
                if mc < 8:
                    dr_chain(mm, wdq_sb, mc * 128, x_sb, 8)
                else:
                    dr_chain(mm, wkv_sb, (mc - 8) * 128, x_sb, 8)
                sq = sq_p.tile([128, TW], bf16, tag=f"sq{mc % 2}")
                nc.scalar.square(sq, mm)
                sq_q.append(sq)
                dpop()
                if mc >= 1:
                    k = mc - 1
                    sqd = sq_q[k]
                    if k < 8:
                        nc.tensor.matmul(sum_q[0:1, :], lhsT=onesb[:, 0:1],
                                         rhs=sqd, start=(k == 0),
                                         stop=(k == 7))
                    else:
                        nc.tensor.matmul(sum_k[0:1, :],
                                         lhsT=twosb[:, 0:1], rhs=sqd,
                                         start=(k == 8), stop=False)
            sqd = sq_q[11]
            nc.tensor.matmul(sum_k[0:1, :], lhsT=twosb[:, 0:1],
                             rhs=sqd, start=False, stop=True)
            # kr chain (no norm dependency)
            mm = psAB.tile([128, TW], f32, tag="m")
            dr_chain(mm, wfk_sb, 512, x_sb, 8)
            krt = rope_p.tile([128, TW], bf16, tag="krt")
            nc.scalar.copy(krt, mm)
            t1 = rope_p.tile([128, TW], f32, tag="t1")
            t2 = rope_p.tile([128, TW], f32, tag="t2")
            nc.vector.scalar_tensor_tensor(
                t1[0:64, :], krt[0:64, :], KR_C, cos_sb[0:64, ts],
                op0=ALU.mult, op1=ALU.mult)
            nc.vector.scalar_tensor_tensor(
                t2[0:64, :], krt[64:128, :], KR_C, sin_sb[64:128, ts],
                op0=ALU.mult, op1=ALU.mult)
            nc.gpsimd.tensor_add(kall_sb[0:64, 4, ts], t1[0:64, :],
                                 t2[0:64, :])
            nc.vector.tensor_copy(kall_sb[64:128, 5, ts],
                                  kall_sb[0:64, 4, ts])
            while di[0] < len(dfill):
                dfill[di[0]]()
                di[0] += 1
            # norms: Ln,Ln then Exp,Exp keeps table switches minimal
            n2_row = nrm_p.tile([1, 2 * TW], f32, tag="n2")
            nq_row = n2_row[:, 0:TW]
            nkv_row = n2_row[:, TW:2 * TW]
            nc.scalar.activation(nq_row, sum_q[0:1, :], func=AF.Ln,
                                 scale=LNA_Q, bias=lnbq_sb)
            nc.scalar.activation(nkv_row, sum_k[0:1, :], func=AF.Ln,
                                 scale=LNA_Q, bias=lnbq_sb)
            nc.scalar.activation(n2_row, n2_row, func=AF.Exp, scale=-0.5)
            nq_bc = nrm_p.tile([128, TW], f32, tag="nqbc")
            nc.gpsimd.partition_broadcast(nq_bc, nq_row)
            nkv_bc = nrm_p.tile([128, TW], f32, tag="nkvbc")
            nc.gpsimd.partition_broadcast(nkv_bc, nkv_row)
            x2 = psO.tile([128, TW], f32, tag="o")
            for j in range(4):
                nc.tensor.transpose(x2[:, j:j + 1],
                                    nkv_row[:, j * 128:(j + 1) * 128],
                                    id1_sb)
            nkvT = nrm_p.tile([128, 4], f32, tag="nkvT")
            nc.vector.tensor_copy(nkvT, x2[:, 0:4])
            return nq_bc, nkv_bc, nkvT

        def b_units(t, nq_bc, nkv_bc, nkvT):
            """list of closures, each one PE chain of phase B for tile t"""
            ts = slice(t * TW, (t + 1) * TW)
            x_sb = x8_tiles[t]
            xb_sb = xb_tiles[t]
            units = []
            holder = {}
            qall_sb = qall_p.tile([128, 6, TW], fp8, tag="qall")

            def qc_unit(h):
                def f():
                    mm = psAB.tile([128, TW], f32, tag="m")
                    dr_chain(mm, wfq_sb, h * 128, x_sb, 8)
                    nc.vector.tensor_mul(qall_sb[:, h, :], mm, nq_bc)
                return f

            def qprep_unit():
                def f():
                    cosn = rope_p.tile([128, TW], f32, tag="cosn")
                    sinn = rope_p.tile([128, TW], f32, tag="sinn")
                    nc.vector.tensor_mul(cosn, cos_sb[:, ts], nq_bc)
                    nc.vector.tensor_mul(sinn, sin_sb[:, ts], nq_bc)
                    holder["cosn"] = cosn
                    holder["sinn"] = sinn
                return f

            def qr_unit(j):
                def f():
                    qr_ps = psAB.tile([128, TW], f32, tag="m")
                    dr_chain(qr_ps, wfq_sb, 512 + j * 128, x_sb, 8)
                    rot_ps = psAB.tile([128, TW], f32, tag="m")
                    dr_chain(rot_ps, wfq_sb, 768 + j * 128, x_sb, 8)
                    t1 = rope_p.tile([128, TW], f32, tag="t1")
                    t2 = rope_p.tile([128, TW], f32, tag="t2")
                    nc.vector.tensor_mul(t1, qr_ps, holder["cosn"])
                    nc.vector.tensor_mul(t2, rot_ps, holder["sinn"])
                    nc.gpsimd.tensor_add(qall_sb[:, 4 + j, :], t1, t2)
                return f

            def k_unit(h):
                def f():
                    mm = psAB.tile([128, TW], f32, tag="m")
                    dr_chain(mm, wfk_sb, h * 128, x_sb, 8)
                    nc.vector.tensor_mul(kall_sb[:, h, ts], mm, nkv_bc)
                return f

            def v_unit(tc4):
                def f():
                    mm = psAB.tile([128, TW], f32, tag="m")
                    for kt in range(16):
                        nc.tensor.matmul(
                            mm,
                            lhsT=xb_sb[:, kt, tc4 * 128:(tc4 + 1) * 128],
                            rhs=wfv_sb[:, kt, :],
                            start=(kt == 0), stop=(kt == 15))
                    nc.vector.tensor_scalar(
                        v_sb[:, t * 4 + tc4, :, :].rearrange(
                            "p h d -> p (h d)"),
                        mm, nkvT[:, tc4:tc4 + 1], V_C,
                        op0=ALU.mult, op1=ALU.mult)
                return f

            units = [qc_unit(h) for h in range(HL)]
            units.append(qprep_unit())
            units += [qr_unit(j) for j in range(2)]
            units += [k_unit(h) for h in range(HL)]
            units += [v_unit(tc4) for tc4 in range(4)]
            return units, qall_sb

        qall_tiles = {}

        def emit_C(tq, fillers, cad=3):
            """attention for query tile tq with PE filler units woven in"""
            ts = slice(tq * TW, (tq + 1) * TW)
            qall_sb = qall_tiles[tq]
            nkt = 4 * (tq + 1)
            PIPE = 2
            fi = 0
            for h in range(HL):
                qsl = 4 + h // 2 - h
                ksl = 4 + h % 2 - h
                ao_ps = psO.tile([128, TW], f32, tag="o")
                rs_ps = psR.tile([128, TW], f32, tag="r")
                pts = {}
                for kt in range(nkt + PIPE):
                    if kt < nkt:
                        s_ps = psS.tile([128, TW], f32, tag="s")
                        nc.tensor.matmul(
                            s_ps,
                            lhsT=kall_sb[:, h::ksl, kt * 128:(kt + 1) * 128],
                            rhs=qall_sb[:, h::qsl, :],
                            start=True, stop=True, perf_mode=DRM)
                        pt = pt_p.tile([128, TW], bf16, tag="pt")
                        nc.scalar.activation(pt, s_ps, func=AF.Exp,
                                             scale=EXP_SCALE)
                        j = kt - 4 * tq
                        if j >= 0:
                            nc.vector.tensor_mul(pt, pt, mask01[:, j, :])
                        pts[kt] = pt
                    if kt % cad == cad - 1 and fi < len(fillers):
                        fillers[fi]()
                        fi += 1
                    kd = kt - PIPE
                    if kd >= 0:
                        pt = pts.pop(kd)
                        nc.tensor.matmul(
                            ao_ps, lhsT=v_sb[:, kd, h, :], rhs=pt,
                            start=(kd == 0), stop=(kd == nkt - 1))
                        nc.tensor.matmul(
                            rs_ps, lhsT=onesb, rhs=pt,
                            start=(kd == 0), stop=(kd == nkt - 1))
                rbc = rbc_p.tile([128, TW], f32, tag="rbc")
                nc.vector.reciprocal(rbc, rs_ps)
                nc.vector.tensor_mul(ao_sb[:, h, ts], ao_ps, rbc)
            for f in fillers[fi:]:
                f()

        def d_units(tq):
            ts = slice(tq * TW, (tq + 1) * TW)

            def unit(dc):
                def f():
                    mm = psAB.tile([128, TW], f32, tag="m")
                    for kt4 in range(4):
                        nc.tensor.matmul(
                            mm, lhsT=wo_sb[:, kt4, dc * 128:(dc + 1) * 128],
                            rhs=ao_sb[:, kt4, ts],
                            start=(kt4 == 0), stop=(kt4 == 3))
                    o_sb = od_p.tile([128, TW], bf16, tag="od")
                    if dc % 2 == 0:
                        nc.scalar.copy(o_sb, mm)
                    else:
                        nc.vector.tensor_copy(o_sb, mm)
                    nc.gpsimd.dma_start(
                        out=out_d[dc * 128:(dc + 1) * 128, ts], in_=o_sb)
                return f
            return [unit(dc) for dc in range(16)]

        nc.sync.dma_start(out=wdq_sb[:, :, :, 0:512], in_=wdq_d[:, :, :, 0:512])
        load_x(0, xb_too=False)
        prologue()
        load_x(0)
        load_x(1)
        pend_d = []
        for t in range(NT):
            load_x(t + 1)
            nq_bc, nkv_bc, nkvT = emit_A(t, pend_d)
            units, qall_sb = b_units(t, nq_bc, nkv_bc, nkvT)
            qall_tiles[t] = qall_sb
            if t == 0:
                for f in units:
                    f()
            else:
                emit_C(t - 1, units)
                pend_d = d_units(t - 1)
        emit_C(NT - 1, pend_d, cad=6)
        for f in d_units(NT - 1):
            f()

    nc.finalize()
    return nc


def _rope_tables():
    inv_freq = (1.0 / (ROPE_BASE ** (np.arange(0, DR, 2, dtype=np.float64)
                                     / DR)))
    tt = np.arange(T, dtype=np.float64)
    freqs = np.outer(tt, inv_freq)
    emb = np.concatenate([freqs, freqs], axis=-1)
    cos = np.cos(emb).T
    sin = np.sin(emb).T
    cos128 = np.ascontiguousarray(np.concatenate([cos, cos], 0))
    sin128 = np.ascontiguousarray(np.concatenate([sin, sin], 0))
    return cos128, sin128


def _pair_rows(w):
    """(K, N) -> (128, K//256, 2, N) with row index = pr*256 + two*128 + p"""
    K, N = w.shape
    return np.ascontiguousarray(
        w.reshape(K // 256, 2, 128, N).transpose(2, 0, 1, 3))


def _rows(w):
    """(K, N) -> (128, K//128, N) with row index = c*128 + p"""
    K, N = w.shape
    return np.ascontiguousarray(w.reshape(K // 128, 128, N).transpose(1, 0, 2))


def _rot_cols(w, dr):
    K, N = w.shape
    wh = w.reshape(K, N // dr, dr)
    lo, hi = wh[:, :, :dr // 2], wh[:, :, dr // 2:]
    return np.concatenate([-hi, lo], axis=2).reshape(K, N)


def _to_fp8(a):
    import ml_dtypes
    return np.ascontiguousarray(a).astype(ml_dtypes.float8_e4m3)


def _to_bf16(a):
    import ml_dtypes
    return np.ascontiguousarray(a).astype(ml_dtypes.bfloat16)


def kernel(x, W_DQ, W_UQ, W_QR, W_DKV, W_UK, W_UV, W_KR, W_O,
           q_norm_w, kv_norm_w):
    global LAST_EXEC_NS
    from concourse.bass_utils import run_bass_kernel_spmd

    x = np.asarray(x, dtype=np.float64)
    W_DQ = np.asarray(W_DQ, np.float64)
    W_UQ = np.asarray(W_UQ, np.float64)
    W_QR = np.asarray(W_QR, np.float64)
    W_DKV = np.asarray(W_DKV, np.float64)
    W_UK = np.asarray(W_UK, np.float64)
    W_UV = np.asarray(W_UV, np.float64)
    W_KR = np.asarray(W_KR, np.float64)
    W_O = np.asarray(W_O, np.float64)
    q_norm_w = np.asarray(q_norm_w, np.float64)
    kv_norm_w = np.asarray(kv_norm_w, np.float64)

    # fold norm weights into up-projections (host, f64)
    wuq_f = W_UQ * q_norm_w[:, None]
    wqr_f = W_QR * q_norm_w[:, None]
    wuk_f = W_UK * kv_norm_w[:, None]
    wuv_f = W_UV * kv_norm_w[:, None]

    cos128, sin128 = _rope_tables()
    cos_b = _to_bf16(cos128)
    sin_b = _to_bf16(sin128)

    wuq_h = wuq_f.reshape(DQ, H, DH)
    wqr_h = wqr_f.reshape(DQ, H, DR)
    wuk_h = wuk_f.reshape(DKV, H, DH)
    wuv_h = wuv_f.reshape(DKV, H, DH)
    wo_h = W_O.reshape(H, DH, D)

    wdq_p = _to_fp8(_pair_rows(W_DQ * SA))
    wkv_p = _to_fp8(_pair_rows(W_DKV * SA))
    wkr_cat = np.concatenate(
        [W_KR * SKR, _rot_cols(W_KR.reshape(D, DR), DR) * SKR], axis=1)

    in_maps = []
    for ci in range(NCORES):
        b, hg = divmod(ci, H // HL)
        hsl = slice(hg * HL, (hg + 1) * HL)
        wuq_s = wuq_h[:, hsl].reshape(DQ, HL * DH)
        wqr_s = wqr_h[:, hsl].reshape(DQ, HL * DR)
        wfq = np.concatenate(
            [W_DQ @ wuq_s, W_DQ @ wqr_s, W_DQ @ _rot_cols(wqr_s, DR)],
            axis=1) * SFQ                                    # (D, 1024)
        wuk_s = wuk_h[:, hsl].reshape(DKV, HL * DH)
        wfk = np.concatenate(
            [W_DKV @ wuk_s * SFK, wkr_cat], axis=1)          # (D, 640)
        wfv = W_DKV @ wuv_h[:, hsl].reshape(DKV, HL * DH)    # (D, 512)
        in_maps.append({
            "x8": _to_fp8(_pair_rows(x[b].T)),
            "xb": _to_bf16(_rows(x[b].T)),
            "wdq": wdq_p,
            "wkv": wkv_p,
            "wfq": _to_fp8(_pair_rows(wfq)),
            "wfk": _to_fp8(_pair_rows(wfk)),
            "wfv": _to_bf16(_rows(wfv)),
            "wo": _to_bf16(_rows(wo_h[hsl].reshape(HL * DH, D))),
            "costab": cos_b,
            "sintab": sin_b,
        })

    if "nc" not in _CACHE:
        _CACHE["nc"] = _build()
    nc = _CACHE["nc"]

    import os as _os
    _trace = _os.environ.get("MLA_TRACE") == "1"
    res = run_bass_kernel_spmd(
        nc, in_maps, core_ids=list(range(NCORES)), trace=_trace)
    LAST_EXEC_NS = res.exec_time_ns
    outs = [res.results[ci]["final_t"] for ci in range(NCORES)]

    out = np.zeros((B, T, D), np.float32)
    for ci in range(NCORES):
        b = ci // (H // HL)
        out[b] += np.asarray(outs[ci], dtype=np.float32).T
    return out


# revision 7
# speedup vs baseline: 1.0312x; 1.0027x over previous
"""Multi-Head Latent Attention (MLA) forward on 8 Trainium2 NeuronCores, v3.

Problem shapes (hardcoded, self-contained):
  B=2, T=2048, D=2048, H=16, DH=128, DKV=512, DQ=1024, DR=64, fp32 I/O.

Sharding: core ci = b*4 + hg; each core owns one batch element and 4 heads.
Up-projection weights sharded over heads; W_O input-dim sharded, each core
emits a partial (D,T) output summed on host.

Mixed-precision strategy (validated against an element-exact numpy model):
  * Q/K score inputs tolerate fp8-e4m3 noise (softmax diversifies it), so
    the entire Q and K paths run as HOST-FUSED single matmuls
    x @ (W_DQ@W_UQ), x @ (W_DKV@W_UK), x @ W_KR in fp8 DoubleRow mode
    (two 128-row k-tiles per instruction at 0.5 cycles/output element).
    Fusing means each path takes ONE fp8 dot-noise instead of two.
  * The V path, attention probabilities, attention output, and the final
    W_O projection stay bf16 end-to-end: peaked causal attention passes
    V-path noise straight to the output max-error metric.
  * RMSNorm denominators come from dedicated fp8 DoubleRow chains
    (x@W_DQ, x@W_DKV) whose only consumers are sums of squares; the
    rsqrt is a single ACT Rsqrt with all fp8/host scales folded into its
    scale/bias constants.  Norm multipliers are folded into the psum->sbuf
    quantize ops (never a separate pass).
  * Rotate-half is produced by matmul against host-permuted weight columns.
  * Causal masking multiplies exp outputs by a 0/1 bf16 mask on Pool,
    off the PSUM critical path.  Row-sums use an all-ones bf16 lhsT whose
    output lands pre-broadcast across all 128 partitions.
"""

import math

import numpy as np

B, T, D = 2, 2048, 2048
H, DH = 16, 128
DKV, DQ, DR = 512, 1024, 64
ROPE_BASE = 500000.0
EPS = 1e-6
SCALE = 1.0 / math.sqrt(DH + DR)

HL = 4            # heads per core
NCORES = 8
TW = 512          # token tile width
NT = T // TW      # 4 token tiles

# fp8 scale plan (powers of two)
SA = 16.0         # W_DQ / W_DKV norm-chain quantization scale
SFQ = 32.0        # fused Q weight scale
SFK = 32.0        # fused K weight scale
SKR = 16.0        # W_KR scale (columns inside the fused K tensor)
SQ = 2.0          # q/k value scale in fp8
EXP_SCALE = SCALE / (SQ * SQ)
LNA_Q = (SFQ / SQ) ** 2 / (SA * SA * DQ)
LNB_Q = (SFQ / SQ) ** 2 * EPS
LNA_K = (SFK / SQ) ** 2 / (SA * SA * DKV)
LNB_K = (SFK / SQ) ** 2 * EPS
KR_C = SQ / SKR
V_C = SFK / SQ    # v multiplier: nkv_row * V_C = true rsqrt(ms+eps)

_CACHE: dict = {}
LAST_EXEC_NS = None


def _build():
    from contextlib import ExitStack

    import concourse.mybir as mybir
    import concourse.tile as tile
    from concourse.bacc import Bacc

    f32 = mybir.dt.float32
    bf16 = mybir.dt.bfloat16
    fp8 = mybir.dt.float8e4
    AF = mybir.ActivationFunctionType
    DRM = mybir.MatmulPerfMode.DoubleRow
    ALU = mybir.AluOpType

    nc = Bacc("TRN2")

    # fp8 tensors arrive pre-paired: [128, npair, 2, N], row = pr*256+two*128+p
    x8_d = nc.dram_tensor("x8", (128, 8, 2, T), fp8, kind="ExternalInput")
    xb_d = nc.dram_tensor("xb", (128, 16, T), bf16, kind="ExternalInput")
    wdq_d = nc.dram_tensor("wdq", (128, 8, 2, DQ), fp8, kind="ExternalInput")
    wkv_d = nc.dram_tensor("wkv", (128, 8, 2, DKV), fp8, kind="ExternalInput")
    wfq_d = nc.dram_tensor("wfq", (128, 8, 2, 1024), fp8, kind="ExternalInput")
    wfk_d = nc.dram_tensor("wfk", (128, 8, 2, 640), fp8, kind="ExternalInput")
    wfv_d = nc.dram_tensor("wfv", (128, 16, 512), bf16, kind="ExternalInput")
    wo_d = nc.dram_tensor("wo", (128, 4, D), bf16, kind="ExternalInput")
    cos_d = nc.dram_tensor("costab", (128, T), bf16, kind="ExternalInput")
    sin_d = nc.dram_tensor("sintab", (128, T), bf16, kind="ExternalInput")
    out_d = nc.dram_tensor("final_t", (D, T), bf16, kind="ExternalOutput")

    with tile.TileContext(nc) as tc, ExitStack() as ctx:
        pers = ctx.enter_context(tc.tile_pool(name="pers", bufs=1))

        # ---------- persistent weights (DMA in use order) ----------
        wdq_sb = pers.tile([128, 8, 2, DQ], fp8, tag="wdq")
        wkv_sb = pers.tile([128, 8, 2, DKV], fp8, tag="wkv")
        wfq_sb = pers.tile([128, 8, 2, 1024], fp8, tag="wfq")
        wfk_sb = pers.tile([128, 8, 2, 640], fp8, tag="wfk")
        wfv_sb = pers.tile([128, 16, 512], bf16, tag="wfv")
        wo_sb = pers.tile([128, 4, D], bf16, tag="wo")
        cos_sb = pers.tile([128, T], bf16, tag="cos")
        sin_sb = pers.tile([128, T], bf16, tag="sin")

        # ---------- persistent activations ----------
        # kall slots: 0-3 content per head, 4 [kr;0], 5 [0;kr]   (fp8, SQ*khat)
        kall_sb = pers.tile([128, 6, T], fp8, tag="kall")
        # v token-major bf16: [128 tok, kt 16, head, dh]
        v_sb = pers.tile([128, 16, HL, DH], bf16, tag="v")
        # attention out feature-major bf16: [128 dh, head, T]
        ao_sb = pers.tile([128, HL, T], bf16, tag="ao")

        # ---------- constants ----------
        onesb = pers.tile([128, 128], bf16, tag="onesb")
        twosb = pers.tile([128, 1], bf16, tag="twosb")
        lnbq_sb = pers.tile([1, 1], f32, tag="lnbq")
        lnbk_sb = pers.tile([1, 1], f32, tag="lnbk")
        id1_sb = pers.tile([1, 1], f32, tag="id1")
        mask01 = pers.tile([128, 4, TW], bf16, tag="mask01")

        def prologue():
            nc.sync.dma_start(out=wdq_sb[:, :, :, 512:1024], in_=wdq_d[:, :, :, 512:1024])
            nc.sync.dma_start(out=wfq_sb, in_=wfq_d[:, :, :, :])
            nc.sync.dma_start(out=wkv_sb, in_=wkv_d[:, :, :, :])
            nc.sync.dma_start(out=wfk_sb, in_=wfk_d[:, :, :, :])
            nc.sync.dma_start(out=wfv_sb, in_=wfv_d[:, :, :])
            nc.sync.dma_start(out=cos_sb, in_=cos_d[:, :])
            nc.sync.dma_start(out=sin_sb, in_=sin_d[:, :])
            nc.sync.dma_start(out=wo_sb, in_=wo_d[:, :, :])
            nc.gpsimd.memset(kall_sb[:, 4:6, :], 0.0)
            nc.gpsimd.memset(onesb, 1.0)
            nc.gpsimd.memset(twosb, 2.0)
            nc.vector.memset(lnbq_sb, LNB_Q)
            nc.vector.memset(lnbk_sb, LNB_K)
            nc.vector.memset(id1_sb, 1.0)
            nc.gpsimd.memset(mask01, 1.0)
            for j in range(4):
                nc.gpsimd.affine_select(
                    out=mask01[:, j, :], in_=mask01[:, j, :],
                    compare_op=ALU.is_ge, fill=0.0,
                    base=-128 * j, pattern=[[1, TW]], channel_multiplier=-1)


        # ---------- pools ----------
        x8in = ctx.enter_context(tc.tile_pool(name="x8in", bufs=2))
        xbin = ctx.enter_context(tc.tile_pool(name="xbin", bufs=1))
        qall_p = ctx.enter_context(tc.tile_pool(name="qall", bufs=2))
        sq_p = ctx.enter_context(tc.tile_pool(name="sq", bufs=1))
        nrm_p = ctx.enter_context(tc.tile_pool(name="nrm", bufs=1))
        rope_p = ctx.enter_context(tc.tile_pool(name="rope", bufs=1))
        pt_p = ctx.enter_context(tc.tile_pool(name="pt", bufs=4))
        rbc_p = ctx.enter_context(tc.tile_pool(name="rbc", bufs=1))
        od_p = ctx.enter_context(tc.tile_pool(name="od", bufs=6))

        psAB = ctx.enter_context(tc.tile_pool(name="psAB", bufs=4, space="PSUM"))
        psS = ctx.enter_context(tc.tile_pool(name="psS", bufs=2, space="PSUM"))
        psO = ctx.enter_context(tc.tile_pool(name="psO", bufs=1, space="PSUM"))
        psR = ctx.enter_context(tc.tile_pool(name="psR", bufs=1, space="PSUM"))

        def dr_chain(out_ps, w_sb, col0, x_sb, npair):
            for pr in range(npair):
                nc.tensor.matmul(
                    out_ps,
                    lhsT=w_sb[:, pr, :, col0:col0 + 128],
                    rhs=x_sb[:, pr, :, :],
                    start=(pr == 0), stop=(pr == npair - 1),
                    perf_mode=DRM)

        # ---------- main loop with cross-phase interleaving ----------
        x8_tiles = {}
        xb_tiles = {}

        def load_x(t, xb_too=True):
            ts = slice(t * TW, (t + 1) * TW)
            if t < NT and t not in x8_tiles:
                xs = x8in.tile([128, 8, 2, TW], fp8, tag="x8")
                nc.sync.dma_start(out=xs, in_=x8_d[:, :, :, ts])
                x8_tiles[t] = xs
            if xb_too and t < NT and t not in xb_tiles:
                xbs = xbin.tile([128, 16, TW], bf16, tag="xb")
                nc.sync.dma_start(out=xbs, in_=xb_d[:, :, ts])
                xb_tiles[t] = xbs

        def emit_A(t, dfill):
            """norm chains + kr; returns (nq_bc, nkv_bc, nkvT) tiles"""
            ts = slice(t * TW, (t + 1) * TW)
            x_sb = x8_tiles[t]
            di = [0]

            def dpop():
                if di[0] < len(dfill):
                    dfill[di[0]]()
                    di[0] += 1
            sum_q = psS.tile([128, TW], f32, tag="s")
            sum_k = psS.tile([128, TW], f32, tag="s")
            sq_q = []
            for mc in range(12):
                mm = psAB.tile([128, TW], f32, tag="m")
                if mc < 8:
                    dr_chain(mm, wdq_sb, mc * 128, x_sb, 8)
                else:
                    dr_chain(mm, wkv_sb, (mc - 8) * 128, x_sb, 8)
                sq = sq_p.tile([128, TW], bf16, tag=f"sq{mc % 2}")
                nc.scalar.square(sq, mm)
                sq_q.append(sq)
                dpop()
                if mc >= 1:
                    k = mc - 1
                    sqd = sq_q[k]
                    if k < 8:
                        nc.tensor.matmul(sum_q[0:1, :], lhsT=onesb[:, 0:1],
                                         rhs=sqd, start=(k == 0),
                                         stop=(k == 7))
                    else:
                        nc.tensor.matmul(sum_k[0:1, :],
                                         lhsT=twosb[:, 0:1], rhs=sqd,
                                         start=(k == 8), stop=False)
            sqd = sq_q[11]
            nc.tensor.matmul(sum_k[0:1, :], lhsT=twosb[:, 0:1],
                             rhs=sqd, start=False, stop=True)
            # kr chain (no norm dependency)
            mm = psAB.tile([128, TW], f32, tag="m")
            dr_chain(mm, wfk_sb, 512, x_sb, 8)
            krt = rope_p.tile([128, TW], bf16, tag="krt")
            nc.scalar.copy(krt, mm)
            t1 = rope_p.tile([128, TW], f32, tag="t1")
            t2 = rope_p.tile([128, TW], f32, tag="t2")
            nc.vector.scalar_tensor_tensor(
                t1[0:64, :], krt[0:64, :], KR_C, cos_sb[0:64, ts],
                op0=ALU.mult, op1=ALU.mult)
            nc.vector.scalar_tensor_tensor(
                t2[0:64, :], krt[64:128, :], KR_C, sin_sb[64:128, ts],
                op0=ALU.mult, op1=ALU.mult)
            nc.gpsimd.tensor_add(kall_sb[0:64, 4, ts], t1[0:64, :],
                                 t2[0:64, :])
            nc.vector.tensor_copy(kall_sb[64:128, 5, ts],
                                  kall_sb[0:64, 4, ts])
            while di[0] < len(dfill):
                dfill[di[0]]()
                di[0] += 1
            # norms: Ln,Ln then Exp,Exp keeps table switches minimal
            n2_row = nrm_p.tile([1, 2 * TW], f32, tag="n2")
            nq_row = n2_row[:, 0:TW]
            nkv_row = n2_row[:, TW:2 * TW]
            nc.scalar.activation(nq_row, sum_q[0:1, :], func=AF.Ln,
                                 scale=LNA_Q, bias=lnbq_sb)
            nc.scalar.activation(nkv_row, sum_k[0:1, :], func=AF.Ln,
                                 scale=LNA_Q, bias=lnbq_sb)
            nc.scalar.activation(n2_row, n2_row, func=AF.Exp, scale=-0.5)
            nq_bc = nrm_p.tile([128, TW], f32, tag="nqbc")
            nc.gpsimd.partition_broadcast(nq_bc, nq_row)
            nkv_bc = nrm_p.tile([128, TW], f32, tag="nkvbc")
            nc.gpsimd.partition_broadcast(nkv_bc, nkv_row)
            x2 = psO.tile([128, TW], f32, tag="o")
            for j in range(4):
                nc.tensor.transpose(x2[:, j:j + 1],
                                    nkv_row[:, j * 128:(j + 1) * 128],
                                    id1_sb)
            nkvT = nrm_p.tile([128, 4], f32, tag="nkvT")
            nc.vector.tensor_copy(nkvT, x2[:, 0:4])
            return nq_bc, nkv_bc, nkvT

        def b_units(t, nq_bc, nkv_bc, nkvT):
            """list of closures, each one PE chain of phase B for tile t"""
            ts = slice(t * TW, (t + 1) * TW)
            x_sb = x8_tiles[t]
            xb_sb = xb_tiles[t]
            units = []
            holder = {}
            qall_sb = qall_p.tile([128, 6, TW], fp8, tag="qall")

            def qc_unit(h):
                def f():
                    mm = psAB.tile([128, TW], f32, tag="m")
                    dr_chain(mm, wfq_sb, h * 128, x_sb, 8)
                    nc.vector.tensor_mul(qall_sb[:, h, :], mm, nq_bc)
                return f

            def qprep_unit():
                def f():
                    cosn = rope_p.tile([128, TW], f32, tag="cosn")
                    sinn = rope_p.tile([128, TW], f32, tag="sinn")
                    nc.vector.tensor_mul(cosn, cos_sb[:, ts], nq_bc)
                    nc.vector.tensor_mul(sinn, sin_sb[:, ts], nq_bc)
                    holder["cosn"] = cosn
                    holder["sinn"] = sinn
                return f

            def qr_unit(j):
                def f():
                    qr_ps = psAB.tile([128, TW], f32, tag="m")
                    dr_chain(qr_ps, wfq_sb, 512 + j * 128, x_sb, 8)
                    rot_ps = psAB.tile([128, TW], f32, tag="m")
                    dr_chain(rot_ps, wfq_sb, 768 + j * 128, x_sb, 8)
                    t1 = rope_p.tile([128, TW], f32, tag="t1")
                    t2 = rope_p.tile([128, TW], f32, tag="t2")
                    nc.vector.tensor_mul(t1, qr_ps, holder["cosn"])
                    nc.vector.tensor_mul(t2, rot_ps, holder["sinn"])
                    nc.gpsimd.tensor_add(qall_sb[:, 4 + j, :], t1, t2)
                return f

            def k_unit(h):
                def f():
                    mm = psAB.tile([128, TW], f32, tag="m")
                    dr_chain(mm, wfk_sb, h * 128, x_sb, 8)
                    nc.vector.tensor_mul(kall_sb[:, h, ts], mm, nkv_bc)
                return f

            def v_unit(tc4):
                def f():
                    mm = psAB.tile([128, TW], f32, tag="m")
                    for kt in range(16):
                        nc.tensor.matmul(
                            mm,
                            lhsT=xb_sb[:, kt, tc4 * 128:(tc4 + 1) * 128],
                            rhs=wfv_sb[:, kt, :],
                            start=(kt == 0), stop=(kt == 15))
                    nc.vector.tensor_scalar(
                        v_sb[:, t * 4 + tc4, :, :].rearrange(
                            "p h d -> p (h d)"),
                        mm, nkvT[:, tc4:tc4 + 1], V_C,
                        op0=ALU.mult, op1=ALU.mult)
                return f

            units = [qc_unit(h) for h in range(HL)]
            units.append(qprep_unit())
            units += [qr_unit(j) for j in range(2)]
            units += [k_unit(h) for h in range(HL)]
            units += [v_unit(tc4) for tc4 in range(4)]
            return units, qall_sb

        qall_tiles = {}

        def emit_C(tq, fillers, cad=3):
            """attention for query tile tq with PE filler units woven in"""
            ts = slice(tq * TW, (tq + 1) * TW)
            qall_sb = qall_tiles[tq]
            nkt = 4 * (tq + 1)
            PIPE = 2
            fi = 0
            for h in range(HL):
                qsl = 4 + h // 2 - h
                ksl = 4 + h % 2 - h
                ao_ps = psO.tile([128, TW], f32, tag="o")
                rs_ps = psR.tile([128, TW], f32, tag="r")
                pts = {}
                for kt in range(nkt + PIPE):
                    if kt < nkt:
                        s_ps = psS.tile([128, TW], f32, tag="s")
                        nc.tensor.matmul(
                            s_ps,
                            lhsT=kall_sb[:, h::ksl, kt * 128:(kt + 1) * 128],
                            rhs=qall_sb[:, h::qsl, :],
                            start=True, stop=True, perf_mode=DRM)
                        pt = pt_p.tile([128, TW], bf16, tag="pt")
                        nc.scalar.activation(pt, s_ps, func=AF.Exp,
                                             scale=EXP_SCALE)
                        j = kt - 4 * tq
                        if j >= 0:
                            nc.vector.tensor_mul(pt, pt, mask01[:, j, :])
                        pts[kt] = pt
                    if kt % cad == cad - 1 and fi < len(fillers):
                        fillers[fi]()
                        fi += 1
                    kd = kt - PIPE
                    if kd >= 0:
                        pt = pts.pop(kd)
                        nc.tensor.matmul(
                            ao_ps, lhsT=v_sb[:, kd, h, :], rhs=pt,
                            start=(kd == 0), stop=(kd == nkt - 1))
                        nc.tensor.matmul(
                            rs_ps, lhsT=onesb, rhs=pt,
                            start=(kd == 0), stop=(kd == nkt - 1))
                rbc = rbc_p.tile([128, TW], f32, tag="rbc")
                nc.vector.reciprocal(rbc, rs_ps)
                nc.vector.tensor_mul(ao_sb[:, h, ts], ao_ps, rbc)
            for f in fillers[fi:]:
                f()

        def d_units(tq):
            ts = slice(tq * TW, (tq + 1) * TW)

            def unit(dc):
                def f():
                    mm = psAB.tile([128, TW], f32, tag="m")
                    for kt4 in range(4):
                        nc.tensor.matmul(
                            mm, lhsT=wo_sb[:, kt4, dc * 128:(dc + 1) * 128],
                            rhs=ao_sb[:, kt4, ts],
                            start=(kt4 == 0), stop=(kt4 == 3))
                    o_sb = od_p.tile([128, TW], bf16, tag="od")
                    if dc % 2 == 0:
                        nc.scalar.copy(o_sb, mm)
                    else:
                        nc.vector.tensor_copy(o_sb, mm)
                    nc.gpsimd.dma_start(
                        out=out_d[dc * 128:(dc + 1) * 128, ts], in_=o_sb)
                return f
            return [unit(dc) for dc in range(16)]

        nc.sync.dma_start(out=wdq_sb[:, :, :, 0:512], in_=wdq_d[:, :, :, 0:512])
        load_x(0, xb_too=False)
        prologue()
        load_x(0)
        load_x(1)
        pend_d = []
        for t in range(NT):
            load_x(t + 1)
            nq_bc, nkv_bc, nkvT = emit_A(t, pend_d)
            units, qall_sb = b_units(t, nq_bc, nkv_bc, nkvT)
            qall_tiles[t] = qall_sb
            if t == 0:
                for f in units:
                    f()
            else:
                emit_C(t - 1, units, cad=t)
                pend_d = d_units(t - 1)
        emit_C(NT - 1, pend_d, cad=6)
        for f in d_units(NT - 1):
            f()

    nc.finalize()
    return nc


def _rope_tables():
    inv_freq = (1.0 / (ROPE_BASE ** (np.arange(0, DR, 2, dtype=np.float64)
                                     / DR)))
    tt = np.arange(T, dtype=np.float64)
    freqs = np.outer(tt, inv_freq)
    emb = np.concatenate([freqs, freqs], axis=-1)
    cos = np.cos(emb).T
    sin = np.sin(emb).T
    cos128 = np.ascontiguousarray(np.concatenate([cos, cos], 0))
    sin128 = np.ascontiguousarray(np.concatenate([sin, sin], 0))
    return cos128, sin128


def _pair_rows(w):
    """(K, N) -> (128, K//256, 2, N) with row index = pr*256 + two*128 + p"""
    K, N = w.shape
    return np.ascontiguousarray(
        w.reshape(K // 256, 2, 128, N).transpose(2, 0, 1, 3))


def _rows(w):
    """(K, N) -> (128, K//128, N) with row index = c*128 + p"""
    K, N = w.shape
    return np.ascontiguousarray(w.reshape(K // 128, 128, N).transpose(1, 0, 2))


def _rot_cols(w, dr):
    K, N = w.shape
    wh = w.reshape(K, N // dr, dr)
    lo, hi = wh[:, :, :dr // 2], wh[:, :, dr // 2:]
    return np.concatenate([-hi, lo], axis=2).reshape(K, N)


def _to_fp8(a):
    import ml_dtypes
    return np.ascontiguousarray(a).astype(ml_dtypes.float8_e4m3)


def _to_bf16(a):
    import ml_dtypes
    return np.ascontiguousarray(a).astype(ml_dtypes.bfloat16)


def kernel(x, W_DQ, W_UQ, W_QR, W_DKV, W_UK, W_UV, W_KR, W_O,
           q_norm_w, kv_norm_w):
    global LAST_EXEC_NS
    from concourse.bass_utils import run_bass_kernel_spmd

    x = np.asarray(x, dtype=np.float64)
    W_DQ = np.asarray(W_DQ, np.float64)
    W_UQ = np.asarray(W_UQ, np.float64)
    W_QR = np.asarray(W_QR, np.float64)
    W_DKV = np.asarray(W_DKV, np.float64)
    W_UK = np.asarray(W_UK, np.float64)
    W_UV = np.asarray(W_UV, np.float64)
    W_KR = np.asarray(W_KR, np.float64)
    W_O = np.asarray(W_O, np.float64)
    q_norm_w = np.asarray(q_norm_w, np.float64)
    kv_norm_w = np.asarray(kv_norm_w, np.float64)

    # fold norm weights into up-projections (host, f64)
    wuq_f = W_UQ * q_norm_w[:, None]
    wqr_f = W_QR * q_norm_w[:, None]
    wuk_f = W_UK * kv_norm_w[:, None]
    wuv_f = W_UV * kv_norm_w[:, None]

    cos128, sin128 = _rope_tables()
    cos_b = _to_bf16(cos128)
    sin_b = _to_bf16(sin128)

    wuq_h = wuq_f.reshape(DQ, H, DH)
    wqr_h = wqr_f.reshape(DQ, H, DR)
    wuk_h = wuk_f.reshape(DKV, H, DH)
    wuv_h = wuv_f.reshape(DKV, H, DH)
    wo_h = W_O.reshape(H, DH, D)

    wdq_p = _to_fp8(_pair_rows(W_DQ * SA))
    wkv_p = _to_fp8(_pair_rows(W_DKV * SA))
    wkr_cat = np.concatenate(
        [W_KR * SKR, _rot_cols(W_KR.reshape(D, DR), DR) * SKR], axis=1)

    in_maps = []
    for ci in range(NCORES):
        b, hg = divmod(ci, H // HL)
        hsl = slice(hg * HL, (hg + 1) * HL)
        wuq_s = wuq_h[:, hsl].reshape(DQ, HL * DH)
        wqr_s = wqr_h[:, hsl].reshape(DQ, HL * DR)
        wfq = np.concatenate(
            [W_DQ @ wuq_s, W_DQ @ wqr_s, W_DQ @ _rot_cols(wqr_s, DR)],
            axis=1) * SFQ                                    # (D, 1024)
        wuk_s = wuk_h[:, hsl].reshape(DKV, HL * DH)
        wfk = np.concatenate(
            [W_DKV @ wuk_s * SFK, wkr_cat], axis=1)          # (D, 640)
        wfv = W_DKV @ wuv_h[:, hsl].reshape(DKV, HL * DH)    # (D, 512)
        in_maps.append({
            "x8": _to_fp8(_pair_rows(x[b].T)),
            "xb": _to_bf16(_rows(x[b].T)),
            "wdq": wdq_p,
            "wkv": wkv_p,
            "wfq": _to_fp8(_pair_rows(wfq)),
            "wfk": _to_fp8(_pair_rows(wfk)),
            "wfv": _to_bf16(_rows(wfv)),
            "wo": _to_bf16(_rows(wo_h[hsl].reshape(HL * DH, D))),
            "costab": cos_b,
            "sintab": sin_b,
        })

    if "nc" not in _CACHE:
        _CACHE["nc"] = _build()
    nc = _CACHE["nc"]

    import os as _os
    _trace = _os.environ.get("MLA_TRACE") == "1"
    res = run_bass_kernel_spmd(
        nc, in_maps, core_ids=list(range(NCORES)), trace=_trace)
    LAST_EXEC_NS = res.exec_time_ns
    outs = [res.results[ci]["final_t"] for ci in range(NCORES)]

    out = np.zeros((B, T, D), np.float32)
    for ci in range(NCORES):
        b = ci // (H // HL)
        out[b] += np.asarray(outs[ci], dtype=np.float32).T
    return out


# revision 8
# speedup vs baseline: 1.0414x; 1.0098x over previous
"""Multi-Head Latent Attention (MLA) forward on 8 Trainium2 NeuronCores, v3.

Problem shapes (hardcoded, self-contained):
  B=2, T=2048, D=2048, H=16, DH=128, DKV=512, DQ=1024, DR=64, fp32 I/O.

Sharding: core ci = b*4 + hg; each core owns one batch element and 4 heads.
Up-projection weights sharded over heads; W_O input-dim sharded, each core
emits a partial (D,T) output summed on host.

Mixed-precision strategy (validated against an element-exact numpy model):
  * Q/K score inputs tolerate fp8-e4m3 noise (softmax diversifies it), so
    the entire Q and K paths run as HOST-FUSED single matmuls
    x @ (W_DQ@W_UQ), x @ (W_DKV@W_UK), x @ W_KR in fp8 DoubleRow mode
    (two 128-row k-tiles per instruction at 0.5 cycles/output element).
    Fusing means each path takes ONE fp8 dot-noise instead of two.
  * The V path, attention probabilities, attention output, and the final
    W_O projection stay bf16 end-to-end: peaked causal attention passes
    V-path noise straight to the output max-error metric.
  * RMSNorm denominators come from dedicated fp8 DoubleRow chains
    (x@W_DQ, x@W_DKV) whose only consumers are sums of squares; the
    rsqrt is a single ACT Rsqrt with all fp8/host scales folded into its
    scale/bias constants.  Norm multipliers are folded into the psum->sbuf
    quantize ops (never a separate pass).
  * Rotate-half is produced by matmul against host-permuted weight columns.
  * Causal masking multiplies exp outputs by a 0/1 bf16 mask on Pool,
    off the PSUM critical path.  Row-sums use an all-ones bf16 lhsT whose
    output lands pre-broadcast across all 128 partitions.
"""

import math

import numpy as np

B, T, D = 2, 2048, 2048
H, DH = 16, 128
DKV, DQ, DR = 512, 1024, 64
ROPE_BASE = 500000.0
EPS = 1e-6
SCALE = 1.0 / math.sqrt(DH + DR)

HL = 4            # heads per core
NCORES = 8
TW = 512          # token tile width
NT = T // TW      # 4 token tiles

# fp8 scale plan (powers of two)
SA = 16.0         # W_DQ / W_DKV norm-chain quantization scale
SFQ = 32.0        # fused Q weight scale
SFK = 32.0        # fused K weight scale
SKR = 16.0        # W_KR scale (columns inside the fused K tensor)
SQ = 2.0          # q/k value scale in fp8
EXP_SCALE = SCALE / (SQ * SQ)
LNA_Q = (SFQ / SQ) ** 2 / (SA * SA * DQ)
LNB_Q = (SFQ / SQ) ** 2 * EPS
LNA_K = (SFK / SQ) ** 2 / (SA * SA * DKV)
LNB_K = (SFK / SQ) ** 2 * EPS
KR_C = SQ / SKR
V_C = SFK / SQ    # v multiplier: nkv_row * V_C = true rsqrt(ms+eps)

_CACHE: dict = {}
LAST_EXEC_NS = None


def _build():
    from contextlib import ExitStack

    import concourse.mybir as mybir
    import concourse.tile as tile
    from concourse.bacc import Bacc

    f32 = mybir.dt.float32
    bf16 = mybir.dt.bfloat16
    fp8 = mybir.dt.float8e4
    AF = mybir.ActivationFunctionType
    DRM = mybir.MatmulPerfMode.DoubleRow
    ALU = mybir.AluOpType

    nc = Bacc("TRN2")

    # fp8 tensors arrive pre-paired: [128, npair, 2, N], row = pr*256+two*128+p
    x8_d = nc.dram_tensor("x8", (128, 8, 2, T), fp8, kind="ExternalInput")
    xb_d = nc.dram_tensor("xb", (128, 16, T), bf16, kind="ExternalInput")
    wdq_d = nc.dram_tensor("wdq", (128, 8, 2, DQ), fp8, kind="ExternalInput")
    wkv_d = nc.dram_tensor("wkv", (128, 8, 2, DKV), fp8, kind="ExternalInput")
    wfq_d = nc.dram_tensor("wfq", (128, 8, 2, 1024), fp8, kind="ExternalInput")
    wfk_d = nc.dram_tensor("wfk", (128, 8, 2, 640), fp8, kind="ExternalInput")
    wfv_d = nc.dram_tensor("wfv", (128, 16, 512), bf16, kind="ExternalInput")
    wo_d = nc.dram_tensor("wo", (128, 4, D), bf16, kind="ExternalInput")
    cos_d = nc.dram_tensor("costab", (128, T), bf16, kind="ExternalInput")
    sin_d = nc.dram_tensor("sintab", (128, T), bf16, kind="ExternalInput")
    out_d = nc.dram_tensor("final_t", (D, T), bf16, kind="ExternalOutput")

    with tile.TileContext(nc) as tc, ExitStack() as ctx:
        pers = ctx.enter_context(tc.tile_pool(name="pers", bufs=1))

        # ---------- persistent weights (DMA in use order) ----------
        wdq_sb = pers.tile([128, 8, 2, DQ], fp8, tag="wdq")
        wkv_sb = pers.tile([128, 8, 2, DKV], fp8, tag="wkv")
        wfq_sb = pers.tile([128, 8, 2, 1024], fp8, tag="wfq")
        wfk_sb = pers.tile([128, 8, 2, 640], fp8, tag="wfk")
        wfv_sb = pers.tile([128, 16, 512], bf16, tag="wfv")
        wo_sb = pers.tile([128, 4, D], bf16, tag="wo")
        cos_sb = pers.tile([128, T], bf16, tag="cos")
        sin_sb = pers.tile([128, T], bf16, tag="sin")

        # ---------- persistent activations ----------
        # kall slots: 0-3 content per head, 4 [kr;0], 5 [0;kr]   (fp8, SQ*khat)
        kall_sb = pers.tile([128, 6, T], fp8, tag="kall")
        # v token-major bf16: [128 tok, kt 16, head, dh]
        v_sb = pers.tile([128, 16, HL, DH], bf16, tag="v")
        # attention out feature-major bf16: [128 dh, head, T]
        ao_sb = pers.tile([128, HL, T], bf16, tag="ao")

        # ---------- constants ----------
        onesb = pers.tile([128, 128], bf16, tag="onesb")
        twosb = pers.tile([128, 1], bf16, tag="twosb")
        lnbq_sb = pers.tile([1, 1], f32, tag="lnbq")
        lnbk_sb = pers.tile([1, 1], f32, tag="lnbk")
        id1_sb = pers.tile([1, 1], f32, tag="id1")
        mask01 = pers.tile([128, 4, TW], bf16, tag="mask01")

        def prologue():
            nc.sync.dma_start(out=wdq_sb[:, :, :, 512:1024], in_=wdq_d[:, :, :, 512:1024])
            nc.sync.dma_start(out=wkv_sb, in_=wkv_d[:, :, :, :])
            nc.sync.dma_start(out=wfq_sb, in_=wfq_d[:, :, :, :])
            nc.sync.dma_start(out=cos_sb, in_=cos_d[:, :])
            nc.sync.dma_start(out=sin_sb, in_=sin_d[:, :])
            nc.sync.dma_start(out=wfk_sb, in_=wfk_d[:, :, :, :])
            nc.gpsimd.memset(kall_sb[:, 4:6, :], 0.0)
            nc.gpsimd.memset(onesb, 1.0)
            nc.gpsimd.memset(twosb, 2.0)
            nc.vector.memset(lnbq_sb, LNB_Q)
            nc.vector.memset(lnbk_sb, LNB_K)
            nc.vector.memset(id1_sb, 1.0)
            nc.gpsimd.memset(mask01, 1.0)
            for j in range(4):
                nc.gpsimd.affine_select(
                    out=mask01[:, j, :], in_=mask01[:, j, :],
                    compare_op=ALU.is_ge, fill=0.0,
                    base=-128 * j, pattern=[[1, TW]], channel_multiplier=-1)


        # ---------- pools ----------
        x8in = ctx.enter_context(tc.tile_pool(name="x8in", bufs=2))
        xbin = ctx.enter_context(tc.tile_pool(name="xbin", bufs=1))
        qall_p = ctx.enter_context(tc.tile_pool(name="qall", bufs=2))
        sq_p = ctx.enter_context(tc.tile_pool(name="sq", bufs=1))
        nrm_p = ctx.enter_context(tc.tile_pool(name="nrm", bufs=1))
        rope_p = ctx.enter_context(tc.tile_pool(name="rope", bufs=1))
        pt_p = ctx.enter_context(tc.tile_pool(name="pt", bufs=4))
        rbc_p = ctx.enter_context(tc.tile_pool(name="rbc", bufs=1))
        od_p = ctx.enter_context(tc.tile_pool(name="od", bufs=6))

        psAB = ctx.enter_context(tc.tile_pool(name="psAB", bufs=4, space="PSUM"))
        psS = ctx.enter_context(tc.tile_pool(name="psS", bufs=2, space="PSUM"))
        psO = ctx.enter_context(tc.tile_pool(name="psO", bufs=1, space="PSUM"))
        psR = ctx.enter_context(tc.tile_pool(name="psR", bufs=1, space="PSUM"))

        def dr_chain(out_ps, w_sb, col0, x_sb, npair):
            for pr in range(npair):
                nc.tensor.matmul(
                    out_ps,
                    lhsT=w_sb[:, pr, :, col0:col0 + 128],
                    rhs=x_sb[:, pr, :, :],
                    start=(pr == 0), stop=(pr == npair - 1),
                    perf_mode=DRM)

        # ---------- main loop with cross-phase interleaving ----------
        x8_tiles = {}
        xb_tiles = {}

        def load_x(t, xb_too=True):
            ts = slice(t * TW, (t + 1) * TW)
            if t < NT and t not in x8_tiles:
                xs = x8in.tile([128, 8, 2, TW], fp8, tag="x8")
                nc.sync.dma_start(out=xs, in_=x8_d[:, :, :, ts])
                x8_tiles[t] = xs
            if xb_too and t < NT and t not in xb_tiles:
                xbs = xbin.tile([128, 16, TW], bf16, tag="xb")
                nc.sync.dma_start(out=xbs, in_=xb_d[:, :, ts])
                xb_tiles[t] = xbs

        def emit_A(t, dfill):
            """norm chains + kr; returns (nq_bc, nkv_bc, nkvT) tiles"""
            ts = slice(t * TW, (t + 1) * TW)
            x_sb = x8_tiles[t]
            di = [0]

            def dpop():
                if di[0] < len(dfill):
                    dfill[di[0]]()
                    di[0] += 1
            sum_q = psS.tile([128, TW], f32, tag="s")
            sum_k = psS.tile([128, TW], f32, tag="s")
            sq_q = []
            for mc in range(12):
                mm = psAB.tile([128, TW], f32, tag="m")
                if mc < 8:
                    dr_chain(mm, wdq_sb, mc * 128, x_sb, 8)
                else:
                    dr_chain(mm, wkv_sb, (mc - 8) * 128, x_sb, 8)
                sq = sq_p.tile([128, TW], bf16, tag=f"sq{mc % 2}")
                nc.scalar.square(sq, mm)
                sq_q.append(sq)
                dpop()
                if mc >= 1:
                    k = mc - 1
                    sqd = sq_q[k]
                    if k < 8:
                        nc.tensor.matmul(sum_q[0:1, :], lhsT=onesb[:, 0:1],
                                         rhs=sqd, start=(k == 0),
                                         stop=(k == 7))
                    else:
                        nc.tensor.matmul(sum_k[0:1, :],
                                         lhsT=twosb[:, 0:1], rhs=sqd,
                                         start=(k == 8), stop=False)
            sqd = sq_q[11]
            nc.tensor.matmul(sum_k[0:1, :], lhsT=twosb[:, 0:1],
                             rhs=sqd, start=False, stop=True)
            # kr chain (no norm dependency)
            mm = psAB.tile([128, TW], f32, tag="m")
            dr_chain(mm, wfk_sb, 512, x_sb, 8)
            krt = rope_p.tile([128, TW], bf16, tag="krt")
            nc.scalar.copy(krt, mm)
            t1 = rope_p.tile([128, TW], f32, tag="t1")
            t2 = rope_p.tile([128, TW], f32, tag="t2")
            nc.vector.scalar_tensor_tensor(
                t1[0:64, :], krt[0:64, :], KR_C, cos_sb[0:64, ts],
                op0=ALU.mult, op1=ALU.mult)
            nc.vector.scalar_tensor_tensor(
                t2[0:64, :], krt[64:128, :], KR_C, sin_sb[64:128, ts],
                op0=ALU.mult, op1=ALU.mult)
            nc.gpsimd.tensor_add(kall_sb[0:64, 4, ts], t1[0:64, :],
                                 t2[0:64, :])
            nc.vector.tensor_copy(kall_sb[64:128, 5, ts],
                                  kall_sb[0:64, 4, ts])
            while di[0] < len(dfill):
                dfill[di[0]]()
                di[0] += 1
            # norms: Ln,Ln then Exp,Exp keeps table switches minimal
            n2_row = nrm_p.tile([1, 2 * TW], f32, tag="n2")
            nq_row = n2_row[:, 0:TW]
            nkv_row = n2_row[:, TW:2 * TW]
            nc.scalar.activation(nq_row, sum_q[0:1, :], func=AF.Ln,
                                 scale=LNA_Q, bias=lnbq_sb)
            nc.scalar.activation(nkv_row, sum_k[0:1, :], func=AF.Ln,
                                 scale=LNA_Q, bias=lnbq_sb)
            nc.scalar.activation(n2_row, n2_row, func=AF.Exp, scale=-0.5)
            nq_bc = nrm_p.tile([128, TW], f32, tag="nqbc")
            nc.gpsimd.partition_broadcast(nq_bc, nq_row)
            nkv_bc = nrm_p.tile([128, TW], f32, tag="nkvbc")
            nc.gpsimd.partition_broadcast(nkv_bc, nkv_row)
            x2 = psO.tile([128, TW], f32, tag="o")
            for j in range(4):
                nc.tensor.transpose(x2[:, j:j + 1],
                                    nkv_row[:, j * 128:(j + 1) * 128],
                                    id1_sb)
            nkvT = nrm_p.tile([128, 4], f32, tag="nkvT")
            nc.vector.tensor_copy(nkvT, x2[:, 0:4])
            return nq_bc, nkv_bc, nkvT

        def b_units(t, nq_bc, nkv_bc, nkvT):
            """list of closures, each one PE chain of phase B for tile t"""
            ts = slice(t * TW, (t + 1) * TW)
            x_sb = x8_tiles[t]
            xb_sb = xb_tiles[t]
            units = []
            holder = {}
            qall_sb = qall_p.tile([128, 6, TW], fp8, tag="qall")

            def qc_unit(h):
                def f():
                    mm = psAB.tile([128, TW], f32, tag="m")
                    dr_chain(mm, wfq_sb, h * 128, x_sb, 8)
                    nc.vector.tensor_mul(qall_sb[:, h, :], mm, nq_bc)
                return f

            def qprep_unit():
                def f():
                    cosn = rope_p.tile([128, TW], f32, tag="cosn")
                    sinn = rope_p.tile([128, TW], f32, tag="sinn")
                    nc.vector.tensor_mul(cosn, cos_sb[:, ts], nq_bc)
                    nc.vector.tensor_mul(sinn, sin_sb[:, ts], nq_bc)
                    holder["cosn"] = cosn
                    holder["sinn"] = sinn
                return f

            def qr_unit(j):
                def f():
                    qr_ps = psAB.tile([128, TW], f32, tag="m")
                    dr_chain(qr_ps, wfq_sb, 512 + j * 128, x_sb, 8)
                    rot_ps = psAB.tile([128, TW], f32, tag="m")
                    dr_chain(rot_ps, wfq_sb, 768 + j * 128, x_sb, 8)
                    t1 = rope_p.tile([128, TW], f32, tag="t1")
                    t2 = rope_p.tile([128, TW], f32, tag="t2")
                    nc.vector.tensor_mul(t1, qr_ps, holder["cosn"])
                    nc.vector.tensor_mul(t2, rot_ps, holder["sinn"])
                    nc.gpsimd.tensor_add(qall_sb[:, 4 + j, :], t1, t2)
                return f

            def k_unit(h):
                def f():
                    mm = psAB.tile([128, TW], f32, tag="m")
                    dr_chain(mm, wfk_sb, h * 128, x_sb, 8)
                    nc.vector.tensor_mul(kall_sb[:, h, ts], mm, nkv_bc)
                return f

            def v_unit(tc4):
                def f():
                    mm = psAB.tile([128, TW], f32, tag="m")
                    for kt in range(16):
                        nc.tensor.matmul(
                            mm,
                            lhsT=xb_sb[:, kt, tc4 * 128:(tc4 + 1) * 128],
                            rhs=wfv_sb[:, kt, :],
                            start=(kt == 0), stop=(kt == 15))
                    nc.vector.tensor_scalar(
                        v_sb[:, t * 4 + tc4, :, :].rearrange(
                            "p h d -> p (h d)"),
                        mm, nkvT[:, tc4:tc4 + 1], V_C,
                        op0=ALU.mult, op1=ALU.mult)
                return f

            units = [qc_unit(h) for h in range(HL)]
            units.append(qprep_unit())
            units += [qr_unit(j) for j in range(2)]
            units += [k_unit(h) for h in range(HL)]
            units += [v_unit(tc4) for tc4 in range(4)]
            return units, qall_sb

        qall_tiles = {}

        def emit_C(tq, fillers, cad=3):
            """attention for query tile tq with PE filler units woven in"""
            ts = slice(tq * TW, (tq + 1) * TW)
            qall_sb = qall_tiles[tq]
            nkt = 4 * (tq + 1)
            PIPE = 2
            fi = 0
            for h in range(HL):
                qsl = 4 + h // 2 - h
                ksl = 4 + h % 2 - h
                ao_ps = psO.tile([128, TW], f32, tag="o")
                rs_ps = psR.tile([128, TW], f32, tag="r")
                pts = {}
                for kt in range(nkt + PIPE):
                    if kt < nkt:
                        s_ps = psS.tile([128, TW], f32, tag="s")
                        nc.tensor.matmul(
                            s_ps,
                            lhsT=kall_sb[:, h::ksl, kt * 128:(kt + 1) * 128],
                            rhs=qall_sb[:, h::qsl, :],
                            start=True, stop=True, perf_mode=DRM)
                        pt = pt_p.tile([128, TW], bf16, tag="pt")
                        nc.scalar.activation(pt, s_ps, func=AF.Exp,
                                             scale=EXP_SCALE)
                        j = kt - 4 * tq
                        if j >= 0:
                            nc.vector.tensor_mul(pt, pt, mask01[:, j, :])
                        pts[kt] = pt
                    if kt % cad == cad - 1 and fi < len(fillers):
                        fillers[fi]()
                        fi += 1
                    kd = kt - PIPE
                    if kd >= 0:
                        pt = pts.pop(kd)
                        nc.tensor.matmul(
                            ao_ps, lhsT=v_sb[:, kd, h, :], rhs=pt,
                            start=(kd == 0), stop=(kd == nkt - 1))
                        nc.tensor.matmul(
                            rs_ps, lhsT=onesb, rhs=pt,
                            start=(kd == 0), stop=(kd == nkt - 1))
                rbc = rbc_p.tile([128, TW], f32, tag="rbc")
                nc.vector.reciprocal(rbc, rs_ps)
                nc.vector.tensor_mul(ao_sb[:, h, ts], ao_ps, rbc)
            for f in fillers[fi:]:
                f()

        def d_units(tq):
            ts = slice(tq * TW, (tq + 1) * TW)

            def unit(dc):
                def f():
                    mm = psAB.tile([128, TW], f32, tag="m")
                    for kt4 in range(4):
                        nc.tensor.matmul(
                            mm, lhsT=wo_sb[:, kt4, dc * 128:(dc + 1) * 128],
                            rhs=ao_sb[:, kt4, ts],
                            start=(kt4 == 0), stop=(kt4 == 3))
                    o_sb = od_p.tile([128, TW], bf16, tag="od")
                    if dc % 2 == 0:
                        nc.scalar.copy(o_sb, mm)
                    else:
                        nc.vector.tensor_copy(o_sb, mm)
                    nc.gpsimd.dma_start(
                        out=out_d[dc * 128:(dc + 1) * 128, ts], in_=o_sb)
                return f
            return [unit(dc) for dc in range(16)]

        nc.sync.dma_start(out=wdq_sb[:, :, :, 0:512], in_=wdq_d[:, :, :, 0:512])
        load_x(0, xb_too=False)
        prologue()
        load_x(0)
        nc.sync.dma_start(out=wfv_sb, in_=wfv_d[:, :, :])
        nc.sync.dma_start(out=wo_sb, in_=wo_d[:, :, :])
        load_x(1)
        pend_d = []
        for t in range(NT):
            load_x(t + 1)
            nq_bc, nkv_bc, nkvT = emit_A(t, pend_d)
            units, qall_sb = b_units(t, nq_bc, nkv_bc, nkvT)
            qall_tiles[t] = qall_sb
            if t == 0:
                for f in units:
                    f()
            else:
                emit_C(t - 1, units, cad=t)
                pend_d = d_units(t - 1)
        emit_C(NT - 1, pend_d, cad=6)
        for f in d_units(NT - 1):
            f()

    nc.finalize()
    return nc


def _rope_tables():
    inv_freq = (1.0 / (ROPE_BASE ** (np.arange(0, DR, 2, dtype=np.float64)
                                     / DR)))
    tt = np.arange(T, dtype=np.float64)
    freqs = np.outer(tt, inv_freq)
    emb = np.concatenate([freqs, freqs], axis=-1)
    cos = np.cos(emb).T
    sin = np.sin(emb).T
    cos128 = np.ascontiguousarray(np.concatenate([cos, cos], 0))
    sin128 = np.ascontiguousarray(np.concatenate([sin, sin], 0))
    return cos128, sin128


def _pair_rows(w):
    """(K, N) -> (128, K//256, 2, N) with row index = pr*256 + two*128 + p"""
    K, N = w.shape
    return np.ascontiguousarray(
        w.reshape(K // 256, 2, 128, N).transpose(2, 0, 1, 3))


def _rows(w):
    """(K, N) -> (128, K//128, N) with row index = c*128 + p"""
    K, N = w.shape
    return np.ascontiguousarray(w.reshape(K // 128, 128, N).transpose(1, 0, 2))


def _rot_cols(w, dr):
    K, N = w.shape
    wh = w.reshape(K, N // dr, dr)
    lo, hi = wh[:, :, :dr // 2], wh[:, :, dr // 2:]
    return np.concatenate([-hi, lo], axis=2).reshape(K, N)


def _to_fp8(a):
    import ml_dtypes
    return np.ascontiguousarray(a).astype(ml_dtypes.float8_e4m3)


def _to_bf16(a):
    import ml_dtypes
    return np.ascontiguousarray(a).astype(ml_dtypes.bfloat16)


def kernel(x, W_DQ, W_UQ, W_QR, W_DKV, W_UK, W_UV, W_KR, W_O,
           q_norm_w, kv_norm_w):
    global LAST_EXEC_NS
    from concourse.bass_utils import run_bass_kernel_spmd

    x = np.asarray(x, dtype=np.float64)
    W_DQ = np.asarray(W_DQ, np.float64)
    W_UQ = np.asarray(W_UQ, np.float64)
    W_QR = np.asarray(W_QR, np.float64)
    W_DKV = np.asarray(W_DKV, np.float64)
    W_UK = np.asarray(W_UK, np.float64)
    W_UV = np.asarray(W_UV, np.float64)
    W_KR = np.asarray(W_KR, np.float64)
    W_O = np.asarray(W_O, np.float64)
    q_norm_w = np.asarray(q_norm_w, np.float64)
    kv_norm_w = np.asarray(kv_norm_w, np.float64)

    # fold norm weights into up-projections (host, f64)
    wuq_f = W_UQ * q_norm_w[:, None]
    wqr_f = W_QR * q_norm_w[:, None]
    wuk_f = W_UK * kv_norm_w[:, None]
    wuv_f = W_UV * kv_norm_w[:, None]

    cos128, sin128 = _rope_tables()
    cos_b = _to_bf16(cos128)
    sin_b = _to_bf16(sin128)

    wuq_h = wuq_f.reshape(DQ, H, DH)
    wqr_h = wqr_f.reshape(DQ, H, DR)
    wuk_h = wuk_f.reshape(DKV, H, DH)
    wuv_h = wuv_f.reshape(DKV, H, DH)
    wo_h = W_O.reshape(H, DH, D)

    wdq_p = _to_fp8(_pair_rows(W_DQ * SA))
    wkv_p = _to_fp8(_pair_rows(W_DKV * SA))
    wkr_cat = np.concatenate(
        [W_KR * SKR, _rot_cols(W_KR.reshape(D, DR), DR) * SKR], axis=1)

    in_maps = []
    for ci in range(NCORES):
        b, hg = divmod(ci, H // HL)
        hsl = slice(hg * HL, (hg + 1) * HL)
        wuq_s = wuq_h[:, hsl].reshape(DQ, HL * DH)
        wqr_s = wqr_h[:, hsl].reshape(DQ, HL * DR)
        wfq = np.concatenate(
            [W_DQ @ wuq_s, W_DQ @ wqr_s, W_DQ @ _rot_cols(wqr_s, DR)],
            axis=1) * SFQ                                    # (D, 1024)
        wuk_s = wuk_h[:, hsl].reshape(DKV, HL * DH)
        wfk = np.concatenate(
            [W_DKV @ wuk_s * SFK, wkr_cat], axis=1)          # (D, 640)
        wfv = W_DKV @ wuv_h[:, hsl].reshape(DKV, HL * DH)    # (D, 512)
        in_maps.append({
            "x8": _to_fp8(_pair_rows(x[b].T)),
            "xb": _to_bf16(_rows(x[b].T)),
            "wdq": wdq_p,
            "wkv": wkv_p,
            "wfq": _to_fp8(_pair_rows(wfq)),
            "wfk": _to_fp8(_pair_rows(wfk)),
            "wfv": _to_bf16(_rows(wfv)),
            "wo": _to_bf16(_rows(wo_h[hsl].reshape(HL * DH, D))),
            "costab": cos_b,
            "sintab": sin_b,
        })

    if "nc" not in _CACHE:
        _CACHE["nc"] = _build()
    nc = _CACHE["nc"]

    import os as _os
    _trace = _os.environ.get("MLA_TRACE") == "1"
    res = run_bass_kernel_spmd(
        nc, in_maps, core_ids=list(range(NCORES)), trace=_trace)
    LAST_EXEC_NS = res.exec_time_ns
    outs = [res.results[ci]["final_t"] for ci in range(NCORES)]

    out = np.zeros((B, T, D), np.float32)
    for ci in range(NCORES):
        b = ci // (H // HL)
        out[b] += np.asarray(outs[ci], dtype=np.float32).T
    return out
